# revision 1
# baseline (speedup 1.0000x reference)
"""LSTM regression kernel for 8 Trainium2 NeuronCores.

Model (reference): B=2048, IN=2048, H=1024, T=15 steps, x constant across
steps. Data-parallel over batch: each of the 8 cores handles 256 batch rows.

Device strategy (per core, batch BL=256):
 - Everything kept "transposed": state hT/cT stored as [H, BL] with H on
   partitions (8 chunks of 128), so no per-step transposes are needed.
 - gatesT[4H, BL] = W_hh @ hT accumulated in PSUM over 8 K-chunks, plus one
   extra identity-weight matmul that adds the precomputed xgT tile (this
   replaces a per-tile DVE add of the input-gate contribution).
 - xgT[4H, BL] = W_ihAug @ xAugT computed once at start; biases b_ih+b_hh are
   folded in host-side by augmenting x with a ones-row and W_ih with a bias
   row (padded to a whole 128-row chunk).
 - Activations (sigmoid/tanh) on ScalarE directly from PSUM; cell update on
   VectorE per 128-row h-chunk so it pipelines with the matmuls.
 - Matmul inputs in fp16 (fp32 PSUM accumulate) - all operands here are
   small-range, so fp16's 10-bit mantissa beats bf16 at identical PE speed.
   h kept in fp32 for output and re-cast to fp16 each step.
"""

import os
import numpy as np
import ml_dtypes

try:
    import concourse.bass as bass
except ImportError:  # pragma: no cover
    import sys
    sys.path.insert(0, "/opt/trn_rl_repo")
    import concourse.bass as bass
from concourse import bacc
import concourse.mybir as mybir
import concourse.tile as tile
from concourse.bass_utils import run_bass_kernel_spmd
from concourse.masks import make_identity

F32 = mybir.dt.float32
F16 = mybir.dt.float16
AF = mybir.ActivationFunctionType

T = 15
B, IN, H = 2048, 2048, 1024
NCORES = 8
BL = B // NCORES            # 256 batch rows per core
G4 = 4 * H                  # 4096 gate rows
NM = G4 // 128              # 32 gate m-tiles
NKH = H // 128              # 8 hidden K-chunks
INA = IN + 128              # x augmented with ones row, padded to chunk
NKX = INA // 128            # 17 input K-chunks
INIT = 0.01

LAST_EXEC_NS = None
LAST_RESULTS = None

_cached_nc = None


def _build():
    nc = bacc.Bacc(None, target_bir_lowering=False)
    wih_hi = nc.dram_tensor("wih_hi", [INA, G4], F16, kind="ExternalInput")
    wih_lo = nc.dram_tensor("wih_lo", [INA, G4], F16, kind="ExternalInput")
    whh = nc.dram_tensor("whh", [H, G4], F16, kind="ExternalInput")
    xt_hi = nc.dram_tensor("xt_hi", [INA, BL], F16, kind="ExternalInput")
    xt_lo = nc.dram_tensor("xt_lo", [INA, BL], F16, kind="ExternalInput")
    hs = nc.dram_tensor("hs", [T, 128, NKH * BL], F32, kind="ExternalOutput")

    with tile.TileContext(nc) as tc:
        with (
            tc.tile_pool(name="const", bufs=1) as constp,
            tc.tile_pool(name="wihp", bufs=4) as wihp,
            tc.tile_pool(name="state", bufs=2) as statep,
            tc.tile_pool(name="gates", bufs=3) as gatesp,
            tc.tile_pool(name="psum", bufs=8, space="PSUM") as psump,
        ):
            whh_sb = constp.tile([128, NKH * G4], F16, tag="whh")
            xg_hi = constp.tile([128, NM * BL], F16, tag="xghi")
            xg_lo = constp.tile([128, NM * BL], F16, tag="xglo")
            xth_sb = constp.tile([128, NKX * BL], F16, tag="xth")
            xtl_sb = constp.tile([128, NKX * BL], F16, tag="xtl")
            ident = constp.tile([128, 128], F16, tag="ident")
            make_identity(nc, ident[:, :])

            whh_r = whh[:, :].rearrange("(kc p) m -> kc p m", p=128)
            for kc in range(NKH):
                nc.sync.dma_start(whh_sb[:, kc * G4:(kc + 1) * G4], whh_r[kc])
            xth_r = xt_hi[:, :].rearrange("(kc p) b -> kc p b", p=128)
            xtl_r = xt_lo[:, :].rearrange("(kc p) b -> kc p b", p=128)
            for kc in range(NKX):
                nc.sync.dma_start(xth_sb[:, kc * BL:(kc + 1) * BL], xth_r[kc])
                nc.sync.dma_start(xtl_sb[:, kc * BL:(kc + 1) * BL], xtl_r[kc])

            # ---- xg phase: 4 sweeps, each producing ALL 4 gates for an
            # hc-pair (so recurrent step 0 for hc 0..1 can start after the
            # first sweep and overlap the rest of the xg phase) ----
            for sweep in range(4):
                pstiles = [psump.tile([128, BL], F32, tag="ps", name=f"psxg{i}") for i in range(8)]
                for kc in range(NKX):
                    wth = wihp.tile([128, 1024], F16, tag="wihh", name="wth")
                    wtl = wihp.tile([128, 1024], F16, tag="wihl", name="wtl")
                    src_h = wih_hi[kc * 128:(kc + 1) * 128, :].rearrange(
                        "p (g t c) -> p g t c", g=4, t=4
                    )[:, :, sweep, :]
                    src_l = wih_lo[kc * 128:(kc + 1) * 128, :].rearrange(
                        "p (g t c) -> p g t c", g=4, t=4
                    )[:, :, sweep, :]
                    nc.sync.dma_start(wth[:, :], src_h)
                    nc.sync.dma_start(wtl[:, :], src_l)
                    for ml in range(8):
                        for pi, (wt_, xt_) in enumerate(
                            [(wth, xth_sb), (wth, xtl_sb), (wtl, xth_sb)]
                        ):
                            nc.tensor.matmul(
                                pstiles[ml][:, :],
                                wt_[:, ml * 128:(ml + 1) * 128],
                                xt_[:, kc * BL:(kc + 1) * BL],
                                start=(kc == 0 and pi == 0),
                                stop=(kc == NKX - 1 and pi == 2),
                            )
                for ml in range(8):
                    g_, j_ = ml // 2, ml % 2
                    m = g_ * 8 + sweep * 2 + j_
                    nc.scalar.copy(xg_hi[:, m * BL:(m + 1) * BL], pstiles[ml][:, :])
                    nc.vector.tensor_sub(
                        xg_lo[:, m * BL:(m + 1) * BL],
                        pstiles[ml][:, :],
                        xg_hi[:, m * BL:(m + 1) * BL],
                    )

            # ---- recurrent steps ----
            h_prev = statep.tile([128, NKH * BL], F16, tag="hbf")
            c_prev = statep.tile([128, NKH * BL], F32, tag="c")
            nc.any.memset(h_prev[:, :], INIT)
            nc.any.memset(c_prev[:, :], INIT)

            for t in range(T):
                h_bf = statep.tile([128, NKH * BL], F16, tag="hbf")
                h_f32 = statep.tile([128, NKH * BL], F32, tag="hf")
                c_new = statep.tile([128, NKH * BL], F32, tag="c")
                for hc in range(NKH):
                    gt = []
                    for gi in range(4):
                        m = gi * NKH + hc
                        ps = psump.tile([128, BL], F32, tag="ps", name="psrec")
                        for kc in range(NKH):
                            nc.tensor.matmul(
                                ps[:, :],
                                whh_sb[:, kc * G4 + m * 128: kc * G4 + (m + 1) * 128],
                                h_prev[:, kc * BL:(kc + 1) * BL],
                                start=(kc == 0),
                                stop=False,
                            )
                        nc.tensor.matmul(
                            ps[:, :],
                            ident[:, :],
                            xg_hi[:, m * BL:(m + 1) * BL],
                            start=False,
                            stop=False,
                        )
                        nc.tensor.matmul(
                            ps[:, :],
                            ident[:, :],
                            xg_lo[:, m * BL:(m + 1) * BL],
                            start=False,
                            stop=True,
                        )
                        g = gatesp.tile([128, BL], F32, tag=f"g{gi}", name=f"gate{gi}")
                        fn = AF.Tanh if gi == 2 else AF.Sigmoid
                        nc.scalar.activation(g[:, :], ps[:, :], fn)
                        gt.append(g)
                    sl = slice(hc * BL, (hc + 1) * BL)
                    t0 = gatesp.tile([128, BL], F32, tag="t0")
                    t1 = gatesp.tile([128, BL], F32, tag="t1")
                    th = gatesp.tile([128, BL], F32, tag="th")
                    nc.vector.tensor_mul(t0[:, :], gt[0][:, :], gt[2][:, :])
                    nc.vector.tensor_mul(t1[:, :], gt[1][:, :], c_prev[:, sl])
                    nc.vector.tensor_add(c_new[:, sl], t0[:, :], t1[:, :])
                    nc.scalar.activation(th[:, :], c_new[:, sl], AF.Tanh)
                    nc.vector.tensor_mul(h_f32[:, sl], gt[3][:, :], th[:, :])
                    nc.vector.tensor_copy(h_bf[:, sl], h_f32[:, sl])
                nc.sync.dma_start(hs[t], h_f32[:, :])
                h_prev, c_prev = h_bf, c_new

    nc.compile()
    return nc


def timeline_ns():
    from concourse.timeline_sim import TimelineSim
    nc = _get_nc()
    ts = TimelineSim(nc)
    ts.simulate()
    return ts.time


def _get_nc():
    global _cached_nc
    if _cached_nc is None:
        _cached_nc = _build()
    return _cached_nc


def kernel(x, W_ih, W_hh, b_ih, b_hh):
    global LAST_EXEC_NS, LAST_RESULTS
    nc = _get_nc()
    bf = np.float16
    x = np.asarray(x, np.float32)
    W_ih = np.asarray(W_ih, np.float32)
    W_hh = np.asarray(W_hh, np.float32)
    b_ih = np.asarray(b_ih, np.float32)
    b_hh = np.asarray(b_hh, np.float32)

    def hilo(a):
        hi = a.astype(bf)
        lo = (a - hi.astype(np.float32)).astype(bf)
        return hi, lo

    waug = np.zeros((INA, G4), np.float32)
    waug[:IN] = W_ih.T
    waug[IN] = b_ih + b_hh
    waug_hi, waug_lo = hilo(waug)
    whh_bf = np.ascontiguousarray(W_hh.T).astype(bf)

    in_maps = []
    for c in range(NCORES):
        xa = np.zeros((INA, BL), np.float32)
        xa[:IN] = x[c * BL:(c + 1) * BL].T
        xa[IN] = 1.0
        xa_hi, xa_lo = hilo(xa)
        in_maps.append({
            "wih_hi": waug_hi, "wih_lo": waug_lo, "whh": whh_bf,
            "xt_hi": xa_hi, "xt_lo": xa_lo,
        })

    trace = os.environ.get("LSTM_TRACE") == "1"
    res = run_bass_kernel_spmd(
        nc, in_maps, core_ids=list(range(NCORES)), trace=trace
    )
    LAST_EXEC_NS = res.exec_time_ns
    LAST_RESULTS = res

    out = np.empty((T, B, H), np.float32)
    for c in range(NCORES):
        a = res.results[c]["hs"].reshape(T, 128, NKH, BL)
        out[:, c * BL:(c + 1) * BL, :] = (
            a.transpose(0, 3, 2, 1).reshape(T, BL, H)
        )
    return out



# revision 9
# speedup vs baseline: 1.4521x; 1.4521x over previous
"""LSTM regression kernel for 8 Trainium2 NeuronCores.

Model (reference): B=2048, IN=2048, H=1024, T=15 steps, x constant across
steps. Data-parallel over batch: each of the 8 cores handles 256 batch rows.

Per-core design (BL=256 batch cols, everything kept transposed [rows, BL]):
 - Gate rows are host-permuted hc-major: m-tile m = hc*4 + gi (hc = h-chunk
   0..7, gi = gate i/f/g/o). The 4 gates of h-chunk hc live in 4 consecutive
   m-tiles, so each step is processed as 8 hc-groups of 4 m-tiles; cell
   updates spread evenly across the step and the next step's matmuls (which
   consume h chunks in kc-ascending order) never stall on the previous
   step's tail.
 - xgT[4096, BL] = W_ih^T x computed once at start, single fp16 product
   (hi/lo splitting unnecessary for the 2e-2 error budget), stored f16.
 - Per step: gatesT = W_hh^T h accumulated in PSUM over 8 K-chunks (pure
   8 matmuls per m-tile - no identity-matmul adds). xg is added on the DVE
   (tensor_add reading PSUM), gate bias b_ih+b_hh is folded into the
   ScalarE activation's per-partition bias operand. Cell update on DVE with
   f16 gates (f32 cell state); h is produced directly in f16 for the next
   matmul and DMA'd out per-chunk as f16 (host converts to f32).
 - PSUM: one [128,256] f32 accumulator per bank; hc-group g uses banks
   (4g mod 8)..+3 so group g+1 accumulates while group g drains.
 - Weights are host-packed so every DMA is contiguous with >=2KB
   per-partition rows; W_ih streams per k-chunk overlapped with the xg
   matmuls, W_hh per m-group between them.
"""

import os
import numpy as np
import ml_dtypes

try:
    import concourse.bass as bass
except ImportError:  # pragma: no cover
    import sys
    sys.path.insert(0, "/opt/trn_rl_repo")
    import concourse.bass as bass
from concourse import bacc
import concourse.mybir as mybir
import concourse.tile as tile
from concourse.bass_utils import run_bass_kernel_spmd

F32 = mybir.dt.float32
F16 = mybir.dt.float16
AF = mybir.ActivationFunctionType

T = 15
B, IN, H = 2048, 2048, 1024
NCORES = 8
BL = B // NCORES            # 256 batch rows per core
G4 = 4 * H                  # 4096 gate rows
NM = G4 // 128              # 32 gate m-tiles
NMG = 4                     # m-groups of 8 m-tiles (W DMA granularity)
NKH = H // 128              # 8 hidden K-chunks
NKX = IN // 128             # 16 input K-chunks
INIT = 0.01

LAST_EXEC_NS = None
LAST_RESULTS = None

_cached_nc = None


def _build():
    nc = bacc.Bacc(None, target_bir_lowering=False)
    # [mg][kc][128][1024]: W_ih^T k-chunk rows x this m-group's 8*128 cols
    wih = nc.dram_tensor("wih", [NMG, NKX, 128, 1024], F16, kind="ExternalInput")
    # [mg][128][kc][1024]: W_hh^T, partition-major so the per-mg DMA groups
    # (k c) contiguously per partition row
    whh = nc.dram_tensor("whh", [NMG, 128, NKH, 1024], F16, kind="ExternalInput")
    # x^T partition-major: [128][kc][BL]
    xt = nc.dram_tensor("xt", [128, NKX, BL], F16, kind="ExternalInput")
    bias = nc.dram_tensor("bias", [128, NM], F32, kind="ExternalInput")
    hs = nc.dram_tensor("hs", [T, NKH, 128, BL], F16, kind="ExternalOutput")

    with tile.TileContext(nc) as tc:
        with (
            tc.tile_pool(name="const", bufs=1) as constp,
            tc.tile_pool(name="wihp", bufs=20) as wihp,
            tc.tile_pool(name="hp", bufs=2) as hp,
            tc.tile_pool(name="cp", bufs=2) as cp,
            tc.tile_pool(name="prep", bufs=12) as prep,
            tc.tile_pool(name="gp", bufs=10) as gp,
            tc.tile_pool(name="tp", bufs=8) as tp,
            tc.tile_pool(name="psum", bufs=8, space="PSUM") as psump,
        ):
            whh_sb = constp.tile([128, NKH * G4], F16, tag="whh")
            xt_sb = constp.tile([128, NKX * BL], F16, tag="xt")
            xg_sb = constp.tile([128, NM * BL], F16, tag="xg")
            bias_sb = constp.tile([128, NM], F32, tag="bias")

            # ---- input DMAs, ordered by first use ----
            nc.sync.dma_start(bias_sb[:, :], bias[:, :])
            for q in range(4):  # x^T in 4 chunks for fast PE start
                src = xt[:, 4 * q:4 * q + 4, :].rearrange("p k c -> p (k c)")
                nc.sync.dma_start(
                    xt_sb[:, 4 * q * BL:(4 * q + 4) * BL], src)

            wih_tiles = {}
            for mg in range(NMG):
                for kc in range(NKX):
                    wt = wihp.tile([128, 1024], F16, tag="wih", name="wt")
                    nc.sync.dma_start(wt[:, :], wih[mg, kc])
                    wih_tiles[(mg, kc)] = wt
                src = whh[mg].rearrange("p k c -> p (k c)")
                nc.sync.dma_start(
                    whh_sb[:, mg * 8192:(mg + 1) * 8192], src)

            # ---- initial state ----
            h_prev = hp.tile([128, NKH * BL], F16, tag="h")
            c_prev = cp.tile([128, NKH * BL], F32, tag="c")
            nc.vector.memset(h_prev[:, :], INIT)
            nc.gpsimd.memset(c_prev[:, :], INIT)

            def whh_col(kc, m):
                mg, ml = m // 8, m % 8
                off = mg * 8192 + kc * 1024 + ml * 128
                return whh_sb[:, off:off + 128]

            def rec_matmuls(hc, h_in):
                """32 matmuls accumulating the 4 gate m-tiles of h-chunk hc."""
                ps = [psump.tile([128, BL], F32, tag="ps", name="ps")
                      for _ in range(4)]
                for kc in range(NKH):
                    for gi in range(4):
                        nc.tensor.matmul(
                            ps[gi][:, :], whh_col(kc, 4 * hc + gi),
                            h_in[:, kc * BL:(kc + 1) * BL],
                            start=(kc == 0), stop=(kc == NKH - 1))
                return ps

            def drain_hc(t, hc, ps, h_new, c_new):
                """DVE/ACT/DMA ops turning h-chunk hc's 4 PSUM accumulators
                into h/c chunk hc of step t."""
                gates = []
                for gi in range(4):
                    m = 4 * hc + gi
                    pre = prep.tile([128, BL], F32, tag="pre")
                    nc.vector.tensor_add(
                        pre[:, :], ps[gi][:, :], xg_sb[:, m * BL:(m + 1) * BL])
                    g = gp.tile([128, BL], F16, tag="g", name=f"g{gi}")
                    fn = AF.Tanh if gi == 2 else AF.Sigmoid
                    nc.scalar.activation(g[:, :], pre[:, :], fn,
                                         bias=bias_sb[:, m:m + 1])
                    gates.append(g)
                sl = slice(hc * BL, (hc + 1) * BL)
                t0 = tp.tile([128, BL], F16, tag="t0")
                t1 = tp.tile([128, BL], F32, tag="t1")
                nc.vector.tensor_mul(t0[:, :], gates[0][:, :], gates[2][:, :])
                nc.vector.tensor_mul(t1[:, :], gates[1][:, :], c_prev[:, sl])
                nc.vector.tensor_add(c_new[:, sl], t0[:, :], t1[:, :])
                th = tp.tile([128, BL], F16, tag="th")
                nc.scalar.activation(th[:, :], c_new[:, sl], AF.Tanh)
                nc.vector.tensor_mul(h_new[:, sl], gates[3][:, :], th[:, :])
                nc.sync.dma_start(hs[t, hc], h_new[:, sl])

            # ---- load phase: xg matmuls + step-0 matmuls, interleaved per
            # h-chunk so W_ih/W_hh DMAs overlap PE work ----
            h0_new = hp.tile([128, NKH * BL], F16, tag="h")
            c0_new = cp.tile([128, NKH * BL], F32, tag="c")
            for hc in range(NKH):
                mg = hc // 2
                mlo = 4 * (hc % 2)  # 0 or 4: this hc's cols in the wih tiles
                psx = [psump.tile([128, BL], F32, tag="ps", name="psx")
                       for _ in range(4)]
                for kc in range(NKX):
                    wt = wih_tiles[(mg, kc)]
                    for gi in range(4):
                        nc.tensor.matmul(
                            psx[gi][:, :],
                            wt[:, (mlo + gi) * 128:(mlo + gi + 1) * 128],
                            xt_sb[:, kc * BL:(kc + 1) * BL],
                            start=(kc == 0), stop=(kc == NKX - 1))
                if hc % 2 == 1:  # release streamed W_ih tiles
                    for kc in range(NKX):
                        del wih_tiles[(mg, kc)]
                for gi in range(4):  # xg to SBUF (f16)
                    m = 4 * hc + gi
                    nc.scalar.copy(xg_sb[:, m * BL:(m + 1) * BL],
                                   psx[gi][:, :])
                ps0 = rec_matmuls(hc, h_prev)
                drain_hc(0, hc, ps0, h0_new, c0_new)
            h_prev, c_prev = h0_new, c0_new

            # ---- steps 1..T-1 ----
            for t in range(1, T):
                h_new = hp.tile([128, NKH * BL], F16, tag="h")
                c_new = cp.tile([128, NKH * BL], F32, tag="c")
                for hc in range(NKH):
                    ps = rec_matmuls(hc, h_prev)
                    drain_hc(t, hc, ps, h_new, c_new)
                h_prev, c_prev = h_new, c_new

    nc.compile()
    return nc


def timeline_ns():
    from concourse.timeline_sim import TimelineSim
    nc = _get_nc()
    ts = TimelineSim(nc)
    ts.simulate()
    return ts.time


def _get_nc():
    global _cached_nc
    if _cached_nc is None:
        _cached_nc = _build()
    return _cached_nc


def _perm():
    """Gate-row permutation: new position m*128+rr (m = hc*4+gi) <- original
    gate row gi*1024 + hc*128 + rr."""
    gi, hc, rr = np.meshgrid(np.arange(4), np.arange(NKH), np.arange(128),
                             indexing="ij")
    p = np.empty(G4, np.int64)
    m = hc * 4 + gi
    p[(m * 128 + rr).ravel()] = (gi * 1024 + hc * 128 + rr).ravel()
    return p


def make_inputs(x, W_ih, W_hh, b_ih, b_hh):
    """Host-side packing shared by kernel() and the quick tester."""
    f16 = np.float16
    perm = _perm()
    # W_ih^T cols permuted -> [16 kc, 128, 4 mg, 1024] -> [4, 16, 128, 1024]
    wihP = np.ascontiguousarray(
        W_ih.T[:, perm].reshape(NKX, 128, NMG, 1024).transpose(2, 0, 1, 3)
    ).astype(f16)
    whhP = np.ascontiguousarray(
        W_hh.T[:, perm].reshape(NKH, 128, NMG, 1024).transpose(2, 1, 0, 3)
    ).astype(f16)
    biasP = np.ascontiguousarray(
        (b_ih + b_hh)[perm].reshape(NM, 128).T).astype(np.float32)
    in_maps = []
    for c in range(NCORES):
        xtP = np.ascontiguousarray(
            x[c * BL:(c + 1) * BL].T.reshape(NKX, 128, BL).transpose(1, 0, 2)
        ).astype(f16)
        in_maps.append({"wih": wihP, "whh": whhP, "xt": xtP, "bias": biasP})
    return in_maps


def unpack_out(hs_f16):
    """[T, 8, 128, BL] f16 -> [T, BL, H] f32 for one core."""
    return hs_f16.transpose(0, 3, 1, 2).reshape(T, BL, H).astype(np.float32)


def kernel(x, W_ih, W_hh, b_ih, b_hh):
    global LAST_EXEC_NS, LAST_RESULTS
    nc = _get_nc()
    x = np.asarray(x, np.float32)
    in_maps = make_inputs(x, np.asarray(W_ih, np.float32),
                          np.asarray(W_hh, np.float32),
                          np.asarray(b_ih, np.float32),
                          np.asarray(b_hh, np.float32))
    trace = os.environ.get("LSTM_TRACE") == "1"
    res = run_bass_kernel_spmd(
        nc, in_maps, core_ids=list(range(NCORES)), trace=trace
    )
    LAST_EXEC_NS = res.exec_time_ns
    LAST_RESULTS = res

    out = np.empty((T, B, H), np.float32)
    for c in range(NCORES):
        out[:, c * BL:(c + 1) * BL, :] = unpack_out(res.results[c]["hs"])
    return out


# revision 14
# speedup vs baseline: 1.4713x; 1.0132x over previous
"""LSTM regression kernel for 8 Trainium2 NeuronCores.

Model (reference): B=2048, IN=2048, H=1024, T=15 steps, x constant across
steps. Data-parallel over batch: each of the 8 cores handles 256 batch rows.

Per-core design (BL=256 batch cols, everything kept transposed [rows, BL]):
 - Gate rows are host-permuted hc-major: m-tile m = hc*4 + gi (hc = h-chunk
   0..7, gi = gate i/f/g/o). The 4 gates of h-chunk hc live in 4 consecutive
   m-tiles, so each step is processed as 8 hc-groups of 4 m-tiles; cell
   updates spread evenly across the step and the next step's matmuls (which
   consume h chunks in kc-ascending order) never stall on the previous
   step's tail.
 - xgT[4096, BL] = W_ih^T x computed once at start, single fp16 product
   (hi/lo splitting unnecessary for the 2e-2 error budget), stored f16.
 - Per step: gatesT = W_hh^T h accumulated in PSUM over 8 K-chunks (pure
   8 matmuls per m-tile - no identity-matmul adds). xg is added on the DVE
   (tensor_add reading PSUM), gate bias b_ih+b_hh is folded into the
   ScalarE activation's per-partition bias operand. Cell update on DVE with
   f16 gates (f32 cell state); h is produced directly in f16 for the next
   matmul and DMA'd out per-chunk as f16 (host converts to f32).
 - PSUM: one [128,256] f32 accumulator per bank; hc-group g uses banks
   (4g mod 8)..+3 so group g+1 accumulates while group g drains.
 - Weights are host-packed so every DMA is contiguous with >=2KB
   per-partition rows; W_ih streams per k-chunk overlapped with the xg
   matmuls, W_hh per m-group between them.
"""

import os
import numpy as np
import ml_dtypes

try:
    import concourse.bass as bass
except ImportError:  # pragma: no cover
    import sys
    sys.path.insert(0, "/opt/trn_rl_repo")
    import concourse.bass as bass
from concourse import bacc
import concourse.mybir as mybir
import concourse.tile as tile
from concourse.bass_utils import run_bass_kernel_spmd
from concourse.masks import make_identity

F32 = mybir.dt.float32
F16 = mybir.dt.float16
AF = mybir.ActivationFunctionType

T = 15
B, IN, H = 2048, 2048, 1024
NCORES = 8
BL = B // NCORES            # 256 batch rows per core
G4 = 4 * H                  # 4096 gate rows
NM = G4 // 128              # 32 gate m-tiles
NMG = 4                     # m-groups of 8 m-tiles (W DMA granularity)
NKH = H // 128              # 8 hidden K-chunks
NKX = IN // 128             # 16 input K-chunks
INIT = 0.01

LAST_EXEC_NS = None
LAST_RESULTS = None

_cached_nc = None


def _build():
    nc = bacc.Bacc(None, target_bir_lowering=False)
    # [mg][kc][128][1024]: W_ih^T k-chunk rows x this m-group's 8*128 cols
    wih = nc.dram_tensor("wih", [NMG, NKX, 128, 1024], F16, kind="ExternalInput")
    # [mg][128][kc][1024]: W_hh^T, partition-major so the per-mg DMA groups
    # (k c) contiguously per partition row
    whh = nc.dram_tensor("whh", [NMG, 128, NKH, 1024], F16, kind="ExternalInput")
    # x^T partition-major: [128][kc][BL]
    xt = nc.dram_tensor("xt", [128, NKX, BL], F16, kind="ExternalInput")
    bias = nc.dram_tensor("bias", [128, NM], F32, kind="ExternalInput")
    hs = nc.dram_tensor("hs", [T, NKH, 128, BL], F16, kind="ExternalOutput")

    with tile.TileContext(nc) as tc:
        with (
            tc.tile_pool(name="const", bufs=1) as constp,
            tc.tile_pool(name="wihp", bufs=24) as wihp,
            tc.tile_pool(name="hp", bufs=2) as hp,
            tc.tile_pool(name="cp", bufs=2) as cp,
            tc.tile_pool(name="prep", bufs=12) as prep,
            tc.tile_pool(name="gp", bufs=10) as gp,
            tc.tile_pool(name="tp", bufs=8) as tp,
            tc.tile_pool(name="psum", bufs=8, space="PSUM") as psump,
        ):
            whh_sb = constp.tile([128, NKH * G4], F16, tag="whh")
            xt_sb = constp.tile([128, NKX * BL], F16, tag="xt")
            xg_sb = constp.tile([128, NM * BL], F16, tag="xg")
            bias_sb = constp.tile([128, NM], F32, tag="bias")
            ident = constp.tile([128, 128], F16, tag="ident")
            make_identity(nc, ident[:, :])

            # ---- input DMAs, ordered by first use: W_ih feeds the xg phase
            # immediately; W_hh is only needed once the xg phase ends ----
            nc.sync.dma_start(bias_sb[:, :], bias[:, :])
            for q in range(4):  # x^T in 4 chunks for fast PE start
                src = xt[:, 4 * q:4 * q + 4, :].rearrange("p k c -> p (k c)")
                nc.sync.dma_start(
                    xt_sb[:, 4 * q * BL:(4 * q + 4) * BL], src)

            wih_tiles = {}
            for mg in range(NMG):
                for kc in range(NKX):
                    wt = wihp.tile([128, 1024], F16, tag="wih", name="wt")
                    nc.sync.dma_start(wt[:, :], wih[mg, kc])
                    wih_tiles[(mg, kc)] = wt
            for mg in range(NMG):
                src = whh[mg].rearrange("p k c -> p (k c)")
                nc.sync.dma_start(
                    whh_sb[:, mg * 8192:(mg + 1) * 8192], src)

            # ---- initial state ----
            h_prev = hp.tile([128, NKH * BL], F16, tag="h")
            c_prev = cp.tile([128, NKH * BL], F32, tag="c")
            nc.vector.memset(h_prev[:, :], INIT)
            nc.gpsimd.memset(c_prev[:, :], INIT)

            def whh_col(kc, m):
                mg, ml = m // 8, m % 8
                off = mg * 8192 + kc * 1024 + ml * 128
                return whh_sb[:, off:off + 128]

            GATE_ORDER = (2, 0, 1, 3)  # g first: t0 = i*g starts earliest

            def rec_matmuls(hc, h_in):
                """Matmuls accumulating the 4 gate m-tiles of h-chunk hc.
                For the step's last chunk (hc 7) the xg add is done on the PE
                (identity matmul) so the drain skips the DVE pre-add - that
                chain is what the next step's kc=7 matmuls wait on."""
                ps = [psump.tile([128, BL], F32, tag="ps", name="ps")
                      for _ in range(4)]
                last = hc == NKH - 1
                for kc in range(NKH):
                    for gi in range(4):
                        nc.tensor.matmul(
                            ps[gi][:, :], whh_col(kc, 4 * hc + gi),
                            h_in[:, kc * BL:(kc + 1) * BL],
                            start=(kc == 0),
                            stop=(kc == NKH - 1 and not last))
                if last:
                    for gi in GATE_ORDER:
                        m = 4 * hc + gi
                        nc.tensor.matmul(
                            ps[gi][:, :], ident[:, :],
                            xg_sb[:, m * BL:(m + 1) * BL],
                            start=False, stop=True)
                return ps

            def drain_hc(t, hc, ps, h_new, c_new):
                """DVE/ACT/DMA ops turning h-chunk hc's 4 PSUM accumulators
                into h/c chunk hc of step t."""
                last = hc == NKH - 1
                gates = {}
                if last:  # xg already added on PE; activate from PSUM
                    for gi in GATE_ORDER:
                        m = 4 * hc + gi
                        g = gp.tile([128, BL], F16, tag="g", name=f"g{gi}")
                        fn = AF.Tanh if gi == 2 else AF.Sigmoid
                        nc.scalar.activation(g[:, :], ps[gi][:, :], fn,
                                             bias=bias_sb[:, m:m + 1])
                        gates[gi] = g
                else:
                    pres = {}
                    for gi in GATE_ORDER:
                        m = 4 * hc + gi
                        pre = prep.tile([128, BL], F32, tag="pre")
                        nc.vector.tensor_add(
                            pre[:, :], ps[gi][:, :],
                            xg_sb[:, m * BL:(m + 1) * BL])
                        pres[gi] = pre
                    for gi in GATE_ORDER:
                        m = 4 * hc + gi
                        g = gp.tile([128, BL], F16, tag="g", name=f"g{gi}")
                        fn = AF.Tanh if gi == 2 else AF.Sigmoid
                        nc.scalar.activation(g[:, :], pres[gi][:, :], fn,
                                             bias=bias_sb[:, m:m + 1])
                        gates[gi] = g
                sl = slice(hc * BL, (hc + 1) * BL)
                t0 = tp.tile([128, BL], F16, tag="t0")
                t1 = tp.tile([128, BL], F32, tag="t1")
                nc.vector.tensor_mul(t0[:, :], gates[0][:, :], gates[2][:, :])
                nc.vector.tensor_mul(t1[:, :], gates[1][:, :], c_prev[:, sl])
                nc.vector.tensor_add(c_new[:, sl], t0[:, :], t1[:, :])
                th = tp.tile([128, BL], F16, tag="th")
                nc.scalar.activation(th[:, :], c_new[:, sl], AF.Tanh)
                nc.vector.tensor_mul(h_new[:, sl], gates[3][:, :], th[:, :])
                nc.sync.dma_start(hs[t, hc], h_new[:, sl])

            # ---- xg phase: xg = W_ih^T x, streamed against the W_ih DMAs.
            # W_hh arrives during this phase and its tail overlaps step 0.
            for hc in range(NKH):
                mg = hc // 2
                mlo = 4 * (hc % 2)  # 0 or 4: this hc's cols in the wih tiles
                psx = [psump.tile([128, BL], F32, tag="ps", name="psx")
                       for _ in range(4)]
                for kc in range(NKX):
                    wt = wih_tiles[(mg, kc)]
                    for gi in range(4):
                        nc.tensor.matmul(
                            psx[gi][:, :],
                            wt[:, (mlo + gi) * 128:(mlo + gi + 1) * 128],
                            xt_sb[:, kc * BL:(kc + 1) * BL],
                            start=(kc == 0), stop=(kc == NKX - 1))
                for gi in range(4):  # xg to SBUF (f16)
                    m = 4 * hc + gi
                    nc.scalar.copy(xg_sb[:, m * BL:(m + 1) * BL],
                                   psx[gi][:, :])

            # ---- steps 0..T-1 ----
            for t in range(T):
                h_new = hp.tile([128, NKH * BL], F16, tag="h")
                c_new = cp.tile([128, NKH * BL], F32, tag="c")
                for hc in range(NKH):
                    ps = rec_matmuls(hc, h_prev)
                    drain_hc(t, hc, ps, h_new, c_new)
                h_prev, c_prev = h_new, c_new

    nc.compile()
    return nc


def timeline_ns():
    from concourse.timeline_sim import TimelineSim
    nc = _get_nc()
    ts = TimelineSim(nc)
    ts.simulate()
    return ts.time


def _get_nc():
    global _cached_nc
    if _cached_nc is None:
        _cached_nc = _build()
    return _cached_nc


def _perm():
    """Gate-row permutation: new position m*128+rr (m = hc*4+gi) <- original
    gate row gi*1024 + hc*128 + rr."""
    gi, hc, rr = np.meshgrid(np.arange(4), np.arange(NKH), np.arange(128),
                             indexing="ij")
    p = np.empty(G4, np.int64)
    m = hc * 4 + gi
    p[(m * 128 + rr).ravel()] = (gi * 1024 + hc * 128 + rr).ravel()
    return p


def make_inputs(x, W_ih, W_hh, b_ih, b_hh):
    """Host-side packing shared by kernel() and the quick tester."""
    f16 = np.float16
    perm = _perm()
    # W_ih^T cols permuted -> [16 kc, 128, 4 mg, 1024] -> [4, 16, 128, 1024]
    wihP = np.ascontiguousarray(
        W_ih.T[:, perm].reshape(NKX, 128, NMG, 1024).transpose(2, 0, 1, 3)
    ).astype(f16)
    whhP = np.ascontiguousarray(
        W_hh.T[:, perm].reshape(NKH, 128, NMG, 1024).transpose(2, 1, 0, 3)
    ).astype(f16)
    biasP = np.ascontiguousarray(
        (b_ih + b_hh)[perm].reshape(NM, 128).T).astype(np.float32)
    in_maps = []
    for c in range(NCORES):
        xtP = np.ascontiguousarray(
            x[c * BL:(c + 1) * BL].T.reshape(NKX, 128, BL).transpose(1, 0, 2)
        ).astype(f16)
        in_maps.append({"wih": wihP, "whh": whhP, "xt": xtP, "bias": biasP})
    return in_maps


def unpack_out(hs_f16):
    """[T, 8, 128, BL] f16 -> [T, BL, H] f32 for one core."""
    return hs_f16.transpose(0, 3, 1, 2).reshape(T, BL, H).astype(np.float32)


def kernel(x, W_ih, W_hh, b_ih, b_hh):
    global LAST_EXEC_NS, LAST_RESULTS
    nc = _get_nc()
    x = np.asarray(x, np.float32)
    in_maps = make_inputs(x, np.asarray(W_ih, np.float32),
                          np.asarray(W_hh, np.float32),
                          np.asarray(b_ih, np.float32),
                          np.asarray(b_hh, np.float32))
    trace = os.environ.get("LSTM_TRACE") == "1"
    res = run_bass_kernel_spmd(
        nc, in_maps, core_ids=list(range(NCORES)), trace=trace
    )
    LAST_EXEC_NS = res.exec_time_ns
    LAST_RESULTS = res

    out = np.empty((T, B, H), np.float32)
    for c in range(NCORES):
        out[:, c * BL:(c + 1) * BL, :] = unpack_out(res.results[c]["hs"])
    return out


# revision 19
# speedup vs baseline: 1.4732x; 1.0013x over previous
"""LSTM regression kernel for 8 Trainium2 NeuronCores.

Model (reference): B=2048, IN=2048, H=1024, T=15 steps, x constant across
steps. Data-parallel over batch: each of the 8 cores handles 256 batch rows.

Per-core design (BL=256 batch cols, everything kept transposed [rows, BL]):
 - Gate rows are host-permuted hc-major: m-tile m = hc*4 + gi (hc = h-chunk
   0..7, gi = gate i/f/g/o). The 4 gates of h-chunk hc live in 4 consecutive
   m-tiles, so each step is processed as 8 hc-groups of 4 m-tiles; cell
   updates spread evenly across the step and the next step's matmuls (which
   consume h chunks in kc-ascending order) never stall on the previous
   step's tail.
 - xgT[4096, BL] = W_ih^T x computed once at start, single fp16 product
   (hi/lo splitting unnecessary for the 2e-2 error budget), stored f16.
 - Per step: gatesT = W_hh^T h accumulated in PSUM over 8 K-chunks (pure
   8 matmuls per m-tile - no identity-matmul adds). xg is added on the DVE
   (tensor_add reading PSUM), gate bias b_ih+b_hh is folded into the
   ScalarE activation's per-partition bias operand. Cell update on DVE with
   f16 gates (f32 cell state); h is produced directly in f16 for the next
   matmul and DMA'd out per-chunk as f16 (host converts to f32).
 - PSUM: one [128,256] f32 accumulator per bank; hc-group g uses banks
   (4g mod 8)..+3 so group g+1 accumulates while group g drains.
 - Weights are host-packed so every DMA is contiguous with >=2KB
   per-partition rows; W_ih streams per k-chunk overlapped with the xg
   matmuls, W_hh per m-group between them.
"""

import os
import numpy as np
import ml_dtypes

try:
    import concourse.bass as bass
except ImportError:  # pragma: no cover
    import sys
    sys.path.insert(0, "/opt/trn_rl_repo")
    import concourse.bass as bass
from concourse import bacc
import concourse.mybir as mybir
import concourse.tile as tile
from concourse.bass_utils import run_bass_kernel_spmd
from concourse.masks import make_identity

F32 = mybir.dt.float32
F16 = mybir.dt.float16
AF = mybir.ActivationFunctionType

T = 15
B, IN, H = 2048, 2048, 1024
NCORES = 8
BL = B // NCORES            # 256 batch rows per core
G4 = 4 * H                  # 4096 gate rows
NM = G4 // 128              # 32 gate m-tiles
NMG = 4                     # m-groups of 8 m-tiles (W DMA granularity)
NKH = H // 128              # 8 hidden K-chunks
NKX = IN // 128             # 16 input K-chunks
INIT = 0.01

LAST_EXEC_NS = None
LAST_RESULTS = None

_cached_nc = None


def _build():
    nc = bacc.Bacc(None, target_bir_lowering=False)
    # [mg][kc][128][1024]: W_ih^T k-chunk rows x this m-group's 8*128 cols
    wih = nc.dram_tensor("wih", [NMG, NKX, 128, 1024], F16, kind="ExternalInput")
    # [mg][128][kc][1024]: W_hh^T, partition-major so the per-mg DMA groups
    # (k c) contiguously per partition row
    whh = nc.dram_tensor("whh", [NMG, 128, NKH, 1024], F16, kind="ExternalInput")
    # x^T partition-major: [128][kc][BL]
    xt = nc.dram_tensor("xt", [128, NKX, BL], F16, kind="ExternalInput")
    bias = nc.dram_tensor("bias", [128, NM], F32, kind="ExternalInput")
    hs = nc.dram_tensor("hs", [T, NKH, 128, BL], F16, kind="ExternalOutput")

    with tile.TileContext(nc) as tc:
        with (
            tc.tile_pool(name="const", bufs=1) as constp,
            tc.tile_pool(name="wihp", bufs=24) as wihp,
            tc.tile_pool(name="hp", bufs=2) as hp,
            tc.tile_pool(name="cp", bufs=2) as cp,
            tc.tile_pool(name="prep", bufs=12) as prep,
            tc.tile_pool(name="gp", bufs=10) as gp,
            tc.tile_pool(name="tp", bufs=8) as tp,
            tc.tile_pool(name="psum", bufs=8, space="PSUM") as psump,
        ):
            whh_sb = constp.tile([128, NKH * G4], F16, tag="whh")
            xt_sb = constp.tile([128, NKX * BL], F16, tag="xt")
            xg_sb = constp.tile([128, NM * BL], F16, tag="xg")
            bias_sb = constp.tile([128, NM], F32, tag="bias")
            ident = constp.tile([128, 128], F16, tag="ident")
            make_identity(nc, ident[:, :])

            # ---- input DMAs, ordered by first use: W_ih feeds the xg phase
            # immediately; W_hh is only needed once the xg phase ends ----
            for q in range(4):  # x^T in 4 chunks for fast PE start
                src = xt[:, 4 * q:4 * q + 4, :].rearrange("p k c -> p (k c)")
                nc.sync.dma_start(
                    xt_sb[:, 4 * q * BL:(4 * q + 4) * BL], src)

            wih_tiles = {}
            for mg in range(NMG):
                if mg == NMG - 1:  # tiny; needed at step 0's first gates
                    nc.sync.dma_start(bias_sb[:, :], bias[:, :])
                for kc in range(NKX):
                    wt = wihp.tile([128, 1024], F16, tag="wih", name="wt")
                    nc.sync.dma_start(wt[:, :], wih[mg, kc])
                    wih_tiles[(mg, kc)] = wt
            for mg in range(NMG):
                src = whh[mg].rearrange("p k c -> p (k c)")
                nc.sync.dma_start(
                    whh_sb[:, mg * 8192:(mg + 1) * 8192], src)

            # ---- initial state ----
            h_prev = hp.tile([128, NKH * BL], F16, tag="h")
            c_prev = cp.tile([128, NKH * BL], F32, tag="c")
            nc.vector.memset(h_prev[:, :], INIT)
            nc.gpsimd.memset(c_prev[:, :], INIT)

            def whh_col(kc, m):
                mg, ml = m // 8, m % 8
                off = mg * 8192 + kc * 1024 + ml * 128
                return whh_sb[:, off:off + 128]

            GATE_ORDER = (2, 0, 1, 3)  # g first: t0 = i*g starts earliest

            def rec_matmuls(hc, h_in):
                """Matmuls accumulating the 4 gate m-tiles of h-chunk hc.
                For the step's last chunk (hc 7) the xg add is done on the PE
                (identity matmuls, placed right after kc 0 so the stops stay
                on kc 7) - the drain then skips the DVE pre-add; that chain is
                what the next step's kc=7 matmuls wait on."""
                ps = [psump.tile([128, BL], F32, tag="ps", name="ps")
                      for _ in range(4)]
                last = hc == NKH - 1
                for kc in range(NKH):
                    for gi in range(4):
                        nc.tensor.matmul(
                            ps[gi][:, :], whh_col(kc, 4 * hc + gi),
                            h_in[:, kc * BL:(kc + 1) * BL],
                            start=(kc == 0), stop=(kc == NKH - 1))
                    if kc == 0 and last:
                        for gi in range(4):
                            m = 4 * hc + gi
                            nc.tensor.matmul(
                                ps[gi][:, :], ident[:, :],
                                xg_sb[:, m * BL:(m + 1) * BL],
                                start=False, stop=False)
                return ps

            def drain_gates_cell(t, hc, ps, c_new):
                """Gate activations + cell update for h-chunk hc. The step's
                last chunk activates straight from PSUM (xg added on PE)."""
                last = hc == NKH - 1
                gates = {}
                pres = {}
                if not last:
                    for gi in GATE_ORDER:
                        m = 4 * hc + gi
                        pre = prep.tile([128, BL], F32, tag="pre")
                        nc.vector.tensor_add(
                            pre[:, :], ps[gi][:, :],
                            xg_sb[:, m * BL:(m + 1) * BL])
                        pres[gi] = pre
                for gi in GATE_ORDER:
                    m = 4 * hc + gi
                    g = gp.tile([128, BL], F16, tag="g", name=f"g{gi}")
                    fn = AF.Tanh if gi == 2 else AF.Sigmoid
                    src = ps[gi] if last else pres[gi]
                    nc.scalar.activation(g[:, :], src[:, :], fn,
                                         bias=bias_sb[:, m:m + 1])
                    gates[gi] = g
                sl = slice(hc * BL, (hc + 1) * BL)
                t0 = tp.tile([128, BL], F16, tag="t0")
                t1 = tp.tile([128, BL], F32, tag="t1")
                nc.vector.tensor_mul(t0[:, :], gates[0][:, :], gates[2][:, :])
                nc.vector.tensor_mul(t1[:, :], gates[1][:, :], c_prev[:, sl])
                nc.vector.tensor_add(c_new[:, sl], t0[:, :], t1[:, :])
                return gates

            def drain_fin(t, hc, gates, h_new, c_new):
                """tanh(c), h = o*tanh(c), and the h DMA for chunk hc."""
                sl = slice(hc * BL, (hc + 1) * BL)
                th = tp.tile([128, BL], F16, tag="th")
                nc.scalar.activation(th[:, :], c_new[:, sl], AF.Tanh)
                nc.vector.tensor_mul(h_new[:, sl], gates[3][:, :], th[:, :])
                nc.sync.dma_start(hs[t, hc], h_new[:, sl])

            # ---- xg phase: xg = W_ih^T x, streamed against the W_ih DMAs.
            # W_hh arrives during this phase and its tail overlaps step 0.
            for hc in range(NKH):
                mg = hc // 2
                mlo = 4 * (hc % 2)  # 0 or 4: this hc's cols in the wih tiles
                psx = [psump.tile([128, BL], F32, tag="ps", name="psx")
                       for _ in range(4)]
                for kc in range(NKX):
                    wt = wih_tiles[(mg, kc)]
                    for gi in range(4):
                        nc.tensor.matmul(
                            psx[gi][:, :],
                            wt[:, (mlo + gi) * 128:(mlo + gi + 1) * 128],
                            xt_sb[:, kc * BL:(kc + 1) * BL],
                            start=(kc == 0), stop=(kc == NKX - 1))
                for gi in range(4):  # xg to SBUF (f16)
                    m = 4 * hc + gi
                    nc.scalar.copy(xg_sb[:, m * BL:(m + 1) * BL],
                                   psx[gi][:, :])

            # ---- steps 0..T-1 ----
            # The last two chunks are software-pipelined: hc 7's gates are
            # emitted before hc 6's tanh/h so the ACT queue never head-of-line
            # blocks the chain the next step's kc=7 matmuls wait on.
            for t in range(T):
                h_new = hp.tile([128, NKH * BL], F16, tag="h")
                c_new = cp.tile([128, NKH * BL], F32, tag="c")
                for hc in range(NKH - 2):
                    ps = rec_matmuls(hc, h_prev)
                    gates = drain_gates_cell(t, hc, ps, c_new)
                    drain_fin(t, hc, gates, h_new, c_new)
                ps6 = rec_matmuls(NKH - 2, h_prev)
                gates6 = drain_gates_cell(t, NKH - 2, ps6, c_new)
                ps7 = rec_matmuls(NKH - 1, h_prev)
                gates7 = drain_gates_cell(t, NKH - 1, ps7, c_new)
                drain_fin(t, NKH - 2, gates6, h_new, c_new)
                drain_fin(t, NKH - 1, gates7, h_new, c_new)
                h_prev, c_prev = h_new, c_new

    nc.compile()
    return nc


def timeline_ns():
    from concourse.timeline_sim import TimelineSim
    nc = _get_nc()
    ts = TimelineSim(nc)
    ts.simulate()
    return ts.time


def _get_nc():
    global _cached_nc
    if _cached_nc is None:
        _cached_nc = _build()
    return _cached_nc


def _perm():
    """Gate-row permutation: new position m*128+rr (m = hc*4+gi) <- original
    gate row gi*1024 + hc*128 + rr."""
    gi, hc, rr = np.meshgrid(np.arange(4), np.arange(NKH), np.arange(128),
                             indexing="ij")
    p = np.empty(G4, np.int64)
    m = hc * 4 + gi
    p[(m * 128 + rr).ravel()] = (gi * 1024 + hc * 128 + rr).ravel()
    return p


def make_inputs(x, W_ih, W_hh, b_ih, b_hh):
    """Host-side packing shared by kernel() and the quick tester."""
    f16 = np.float16
    perm = _perm()
    # W_ih^T cols permuted -> [16 kc, 128, 4 mg, 1024] -> [4, 16, 128, 1024]
    wihP = np.ascontiguousarray(
        W_ih.T[:, perm].reshape(NKX, 128, NMG, 1024).transpose(2, 0, 1, 3)
    ).astype(f16)
    whhP = np.ascontiguousarray(
        W_hh.T[:, perm].reshape(NKH, 128, NMG, 1024).transpose(2, 1, 0, 3)
    ).astype(f16)
    biasP = np.ascontiguousarray(
        (b_ih + b_hh)[perm].reshape(NM, 128).T).astype(np.float32)
    in_maps = []
    for c in range(NCORES):
        xtP = np.ascontiguousarray(
            x[c * BL:(c + 1) * BL].T.reshape(NKX, 128, BL).transpose(1, 0, 2)
        ).astype(f16)
        in_maps.append({"wih": wihP, "whh": whhP, "xt": xtP, "bias": biasP})
    return in_maps


def unpack_out(hs_f16):
    """[T, 8, 128, BL] f16 -> [T, BL, H] f32 for one core."""
    return hs_f16.transpose(0, 3, 1, 2).reshape(T, BL, H).astype(np.float32)


def kernel(x, W_ih, W_hh, b_ih, b_hh):
    global LAST_EXEC_NS, LAST_RESULTS
    nc = _get_nc()
    x = np.asarray(x, np.float32)
    in_maps = make_inputs(x, np.asarray(W_ih, np.float32),
                          np.asarray(W_hh, np.float32),
                          np.asarray(b_ih, np.float32),
                          np.asarray(b_hh, np.float32))
    trace = os.environ.get("LSTM_TRACE") == "1"
    res = run_bass_kernel_spmd(
        nc, in_maps, core_ids=list(range(NCORES)), trace=trace
    )
    LAST_EXEC_NS = res.exec_time_ns
    LAST_RESULTS = res

    out = np.empty((T, B, H), np.float32)
    for c in range(NCORES):
        out[:, c * BL:(c + 1) * BL, :] = unpack_out(res.results[c]["hs"])
    return out


# revision 21
# speedup vs baseline: 1.5514x; 1.0531x over previous
"""LSTM regression kernel for 8 Trainium2 NeuronCores.

Model (reference): B=2048, IN=2048, H=1024, T=15 steps, x constant across
steps. Data-parallel over batch: each of the 8 cores handles 256 batch rows.

Per-core design (BL=256 batch cols, everything kept transposed [rows, BL]):
 - Gate rows are host-permuted hc-major: m-tile m = hc*4 + gi (hc = h-chunk
   0..7, gi = gate i/f/g/o). The 4 gates of h-chunk hc live in 4 consecutive
   m-tiles, so each step is processed as 8 hc-groups of 4 m-tiles; cell
   updates spread evenly across the step and the next step's matmuls (which
   consume h chunks in kc-ascending order) never stall on the previous
   step's tail.
 - xgT[4096, BL] = W_ih^T x computed once at start, single fp16 product
   (hi/lo splitting unnecessary for the 2e-2 error budget), stored f16.
 - Per step: gatesT = W_hh^T h accumulated in PSUM over 8 K-chunks (pure
   8 matmuls per m-tile - no identity-matmul adds). xg is added on the DVE
   (tensor_add reading PSUM), gate bias b_ih+b_hh is folded into the
   ScalarE activation's per-partition bias operand. Cell update on DVE with
   f16 gates (f32 cell state); h is produced directly in f16 for the next
   matmul and DMA'd out per-chunk as f16 (host converts to f32).
 - PSUM: one [128,256] f32 accumulator per bank; hc-group g uses banks
   (4g mod 8)..+3 so group g+1 accumulates while group g drains.
 - Weights are host-packed so every DMA is contiguous with >=2KB
   per-partition rows; W_ih streams per k-chunk overlapped with the xg
   matmuls, W_hh per m-group between them.
"""

import os
import numpy as np
import ml_dtypes

try:
    import concourse.bass as bass
except ImportError:  # pragma: no cover
    import sys
    sys.path.insert(0, "/opt/trn_rl_repo")
    import concourse.bass as bass
from concourse import bacc
import concourse.mybir as mybir
import concourse.tile as tile
from concourse.bass_utils import run_bass_kernel_spmd
from concourse.masks import make_identity

F32 = mybir.dt.float32
F16 = mybir.dt.float16
AF = mybir.ActivationFunctionType

T = 15
B, IN, H = 2048, 2048, 1024
NCORES = 8
BL = B // NCORES            # 256 batch rows per core
G4 = 4 * H                  # 4096 gate rows
NM = G4 // 128              # 32 gate m-tiles
NMG = 4                     # m-groups of 8 m-tiles (W DMA granularity)
NKH = H // 128              # 8 hidden K-chunks
NKX = IN // 128             # 16 input K-chunks
INIT = 0.01

LAST_EXEC_NS = None
LAST_RESULTS = None

_cached_nc = None


def _build():
    nc = bacc.Bacc(None, target_bir_lowering=False)
    # [mg][kc][128][1024]: W_ih^T k-chunk rows x this m-group's 8*128 cols
    wih = nc.dram_tensor("wih", [NMG, NKX, 128, 1024], F16, kind="ExternalInput")
    # [mg][128][kc][1024]: W_hh^T, partition-major so the per-mg DMA groups
    # (k c) contiguously per partition row
    whh = nc.dram_tensor("whh", [NMG, 128, NKH, 1024], F16, kind="ExternalInput")
    # x^T partition-major: [128][kc][BL]
    xt = nc.dram_tensor("xt", [128, NKX, BL], F16, kind="ExternalInput")
    bias = nc.dram_tensor("bias", [128, NM], F32, kind="ExternalInput")
    hs = nc.dram_tensor("hs", [T, NKH, 128, BL], F16, kind="ExternalOutput")

    with tile.TileContext(nc) as tc:
        with (
            tc.tile_pool(name="const", bufs=1) as constp,
            tc.tile_pool(name="wihp", bufs=24) as wihp,
            tc.tile_pool(name="hp", bufs=2) as hp,
            tc.tile_pool(name="cp", bufs=2) as cp,
            tc.tile_pool(name="prep", bufs=12) as prep,
            tc.tile_pool(name="gp", bufs=10) as gp,
            tc.tile_pool(name="tp", bufs=8) as tp,
            tc.tile_pool(name="psum", bufs=8, space="PSUM") as psump,
        ):
            whh_sb = constp.tile([128, NKH * G4], F16, tag="whh")
            xt_sb = constp.tile([128, NKX * BL], F16, tag="xt")
            xg_sb = constp.tile([128, NM * BL], F16, tag="xg")
            bias_sb = constp.tile([128, NM], F32, tag="bias")
            ident = constp.tile([128, 128], F16, tag="ident")
            make_identity(nc, ident[:, :])

            # ---- input DMAs, ordered by first use: W_ih feeds the xg phase
            # immediately; W_hh is only needed once the xg phase ends ----
            for q in range(4):  # x^T in 4 chunks for fast PE start
                src = xt[:, 4 * q:4 * q + 4, :].rearrange("p k c -> p (k c)")
                nc.sync.dma_start(
                    xt_sb[:, 4 * q * BL:(4 * q + 4) * BL], src)

            wih_tiles = {}
            for mg in range(NMG):
                if mg == NMG - 1:  # tiny; needed at step 0's first gates
                    nc.sync.dma_start(bias_sb[:, :], bias[:, :])
                for kc in range(NKX):
                    wt = wihp.tile([128, 1024], F16, tag="wih", name="wt")
                    nc.sync.dma_start(wt[:, :], wih[mg, kc])
                    wih_tiles[(mg, kc)] = wt
            for mg in range(NMG):
                src = whh[mg].rearrange("p k c -> p (k c)")
                nc.sync.dma_start(
                    whh_sb[:, mg * 8192:(mg + 1) * 8192], src)

            # ---- initial state ----
            h_prev = hp.tile([128, NKH * BL], F16, tag="h")
            c_prev = cp.tile([128, NKH * BL], F32, tag="c")
            nc.vector.memset(h_prev[:, :], INIT)
            nc.gpsimd.memset(c_prev[:, :], INIT)

            def whh_col(kc, m):
                mg, ml = m // 8, m % 8
                off = mg * 8192 + kc * 1024 + ml * 128
                return whh_sb[:, off:off + 128]

            # Gate order within an hc-group: g first (t1 = f*c only needs f;
            # t0 = i*g needs i and g), o last (only consumed by the final h
            # mul). PSUM stops then arrive staggered through the group's
            # window and the drain chain overlaps the matmuls.
            GATE_ORDER = (2, 1, 0, 3)

            def rec_matmuls(hc, h_in):
                """Matmuls accumulating the 4 gate m-tiles of h-chunk hc.
                hc 0 runs kc-major with kc 7 last because h[7] of the
                previous step lands just after the step boundary; all other
                groups run gi-major so each gate's accumulator completes (and
                drains) as early as possible."""
                ps = [psump.tile([128, BL], F32, tag="ps", name="ps")
                      for _ in range(4)]
                if hc == 0:
                    for kc in range(NKH):
                        for gi in GATE_ORDER:
                            nc.tensor.matmul(
                                ps[gi][:, :], whh_col(kc, 4 * hc + gi),
                                h_in[:, kc * BL:(kc + 1) * BL],
                                start=(kc == 0), stop=(kc == NKH - 1))
                else:
                    for gi in GATE_ORDER:
                        for kc in range(NKH):
                            nc.tensor.matmul(
                                ps[gi][:, :], whh_col(kc, 4 * hc + gi),
                                h_in[:, kc * BL:(kc + 1) * BL],
                                start=(kc == 0), stop=(kc == NKH - 1))
                return ps

            def drain_hc(t, hc, ps, h_new, c_new):
                """DVE/ACT/DMA ops turning h-chunk hc's 4 PSUM accumulators
                into h/c chunk hc of step t."""
                gates = {}
                for gi in GATE_ORDER:
                    m = 4 * hc + gi
                    pre = prep.tile([128, BL], F32, tag="pre")
                    nc.vector.tensor_add(
                        pre[:, :], ps[gi][:, :], xg_sb[:, m * BL:(m + 1) * BL])
                    g = gp.tile([128, BL], F16, tag="g", name=f"g{gi}")
                    fn = AF.Tanh if gi == 2 else AF.Sigmoid
                    nc.scalar.activation(g[:, :], pre[:, :], fn,
                                         bias=bias_sb[:, m:m + 1])
                    gates[gi] = g
                sl = slice(hc * BL, (hc + 1) * BL)
                t0 = tp.tile([128, BL], F16, tag="t0")
                t1 = tp.tile([128, BL], F32, tag="t1")
                nc.vector.tensor_mul(t1[:, :], gates[1][:, :], c_prev[:, sl])
                nc.vector.tensor_mul(t0[:, :], gates[0][:, :], gates[2][:, :])
                nc.vector.tensor_add(c_new[:, sl], t0[:, :], t1[:, :])
                th = tp.tile([128, BL], F16, tag="th")
                nc.scalar.activation(th[:, :], c_new[:, sl], AF.Tanh)
                nc.vector.tensor_mul(h_new[:, sl], gates[3][:, :], th[:, :])
                nc.sync.dma_start(hs[t, hc], h_new[:, sl])

            # ---- xg phase: xg = W_ih^T x, streamed against the W_ih DMAs.
            # W_hh arrives during this phase and its tail overlaps step 0.
            for hc in range(NKH):
                mg = hc // 2
                mlo = 4 * (hc % 2)  # 0 or 4: this hc's cols in the wih tiles
                psx = [psump.tile([128, BL], F32, tag="ps", name="psx")
                       for _ in range(4)]
                for kc in range(NKX):
                    wt = wih_tiles[(mg, kc)]
                    for gi in range(4):
                        nc.tensor.matmul(
                            psx[gi][:, :],
                            wt[:, (mlo + gi) * 128:(mlo + gi + 1) * 128],
                            xt_sb[:, kc * BL:(kc + 1) * BL],
                            start=(kc == 0), stop=(kc == NKX - 1))
                for gi in range(4):  # xg to SBUF (f16)
                    m = 4 * hc + gi
                    nc.scalar.copy(xg_sb[:, m * BL:(m + 1) * BL],
                                   psx[gi][:, :])

            # ---- steps 0..T-1 ----
            for t in range(T):
                h_new = hp.tile([128, NKH * BL], F16, tag="h")
                c_new = cp.tile([128, NKH * BL], F32, tag="c")
                for hc in range(NKH):
                    ps = rec_matmuls(hc, h_prev)
                    drain_hc(t, hc, ps, h_new, c_new)
                h_prev, c_prev = h_new, c_new

    nc.compile()
    return nc


def timeline_ns():
    from concourse.timeline_sim import TimelineSim
    nc = _get_nc()
    ts = TimelineSim(nc)
    ts.simulate()
    return ts.time


def _get_nc():
    global _cached_nc
    if _cached_nc is None:
        _cached_nc = _build()
    return _cached_nc


def _perm():
    """Gate-row permutation: new position m*128+rr (m = hc*4+gi) <- original
    gate row gi*1024 + hc*128 + rr."""
    gi, hc, rr = np.meshgrid(np.arange(4), np.arange(NKH), np.arange(128),
                             indexing="ij")
    p = np.empty(G4, np.int64)
    m = hc * 4 + gi
    p[(m * 128 + rr).ravel()] = (gi * 1024 + hc * 128 + rr).ravel()
    return p


def make_inputs(x, W_ih, W_hh, b_ih, b_hh):
    """Host-side packing shared by kernel() and the quick tester."""
    f16 = np.float16
    perm = _perm()
    # W_ih^T cols permuted -> [16 kc, 128, 4 mg, 1024] -> [4, 16, 128, 1024]
    wihP = np.ascontiguousarray(
        W_ih.T[:, perm].reshape(NKX, 128, NMG, 1024).transpose(2, 0, 1, 3)
    ).astype(f16)
    whhP = np.ascontiguousarray(
        W_hh.T[:, perm].reshape(NKH, 128, NMG, 1024).transpose(2, 1, 0, 3)
    ).astype(f16)
    biasP = np.ascontiguousarray(
        (b_ih + b_hh)[perm].reshape(NM, 128).T).astype(np.float32)
    in_maps = []
    for c in range(NCORES):
        xtP = np.ascontiguousarray(
            x[c * BL:(c + 1) * BL].T.reshape(NKX, 128, BL).transpose(1, 0, 2)
        ).astype(f16)
        in_maps.append({"wih": wihP, "whh": whhP, "xt": xtP, "bias": biasP})
    return in_maps


def unpack_out(hs_f16):
    """[T, 8, 128, BL] f16 -> [T, BL, H] f32 for one core."""
    return hs_f16.transpose(0, 3, 1, 2).reshape(T, BL, H).astype(np.float32)


def kernel(x, W_ih, W_hh, b_ih, b_hh):
    global LAST_EXEC_NS, LAST_RESULTS
    nc = _get_nc()
    x = np.asarray(x, np.float32)
    in_maps = make_inputs(x, np.asarray(W_ih, np.float32),
                          np.asarray(W_hh, np.float32),
                          np.asarray(b_ih, np.float32),
                          np.asarray(b_hh, np.float32))
    trace = os.environ.get("LSTM_TRACE") == "1"
    res = run_bass_kernel_spmd(
        nc, in_maps, core_ids=list(range(NCORES)), trace=trace
    )
    LAST_EXEC_NS = res.exec_time_ns
    LAST_RESULTS = res

    out = np.empty((T, B, H), np.float32)
    for c in range(NCORES):
        out[:, c * BL:(c + 1) * BL, :] = unpack_out(res.results[c]["hs"])
    return out


# revision 24
# speedup vs baseline: 1.5590x; 1.0048x over previous
"""LSTM regression kernel for 8 Trainium2 NeuronCores.

Model (reference): B=2048, IN=2048, H=1024, T=15 steps, x constant across
steps. Data-parallel over batch: each of the 8 cores handles 256 batch rows.

Per-core design (BL=256 batch cols, everything kept transposed [rows, BL]):
 - Gate rows are host-permuted hc-major: m-tile m = hc*4 + gi (hc = h-chunk
   0..7, gi = gate i/f/g/o). The 4 gates of h-chunk hc live in 4 consecutive
   m-tiles, so each step is processed as 8 hc-groups of 4 m-tiles; cell
   updates spread evenly across the step and the next step's matmuls (which
   consume h chunks in kc-ascending order) never stall on the previous
   step's tail.
 - xgT[4096, BL] = W_ih^T x computed once at start, single fp16 product
   (hi/lo splitting unnecessary for the 2e-2 error budget), stored f16.
 - Per step: gatesT = W_hh^T h accumulated in PSUM over 8 K-chunks (pure
   8 matmuls per m-tile - no identity-matmul adds). xg is added on the DVE
   (tensor_add reading PSUM), gate bias b_ih+b_hh is folded into the
   ScalarE activation's per-partition bias operand. Cell update on DVE with
   f16 gates (f32 cell state); h is produced directly in f16 for the next
   matmul and DMA'd out per-chunk as f16 (host converts to f32).
 - PSUM: one [128,256] f32 accumulator per bank; hc-group g uses banks
   (4g mod 8)..+3 so group g+1 accumulates while group g drains.
 - Weights are host-packed so every DMA is contiguous with >=2KB
   per-partition rows; W_ih streams per k-chunk overlapped with the xg
   matmuls, W_hh per m-group between them.
"""

import os
import numpy as np
import ml_dtypes

try:
    import concourse.bass as bass
except ImportError:  # pragma: no cover
    import sys
    sys.path.insert(0, "/opt/trn_rl_repo")
    import concourse.bass as bass
from concourse import bacc
import concourse.mybir as mybir
import concourse.tile as tile
from concourse.bass_utils import run_bass_kernel_spmd

F32 = mybir.dt.float32
F16 = mybir.dt.float16
AF = mybir.ActivationFunctionType

T = 15
B, IN, H = 2048, 2048, 1024
NCORES = 8
BL = B // NCORES            # 256 batch rows per core
G4 = 4 * H                  # 4096 gate rows
NM = G4 // 128              # 32 gate m-tiles
NMG = 4                     # m-groups of 8 m-tiles (W DMA granularity)
NKH = H // 128              # 8 hidden K-chunks
NKX = IN // 128             # 16 input K-chunks
INIT = 0.01

LAST_EXEC_NS = None
LAST_RESULTS = None

_cached_nc = None


def _build():
    nc = bacc.Bacc(None, target_bir_lowering=False)
    # [mg][kc][128][1024]: W_ih^T k-chunk rows x this m-group's 8*128 cols
    wih = nc.dram_tensor("wih", [NMG, NKX, 128, 1024], F16, kind="ExternalInput")
    # [mg][128][kc][1024]: W_hh^T, partition-major so the per-mg DMA groups
    # (k c) contiguously per partition row
    whh = nc.dram_tensor("whh", [NMG, 128, NKH, 1024], F16, kind="ExternalInput")
    # x^T partition-major: [128][kc][BL]
    xt = nc.dram_tensor("xt", [128, NKX, BL], F16, kind="ExternalInput")
    bias = nc.dram_tensor("bias", [128, NM], F32, kind="ExternalInput")
    hs = nc.dram_tensor("hs", [T, NKH, 128, BL], F16, kind="ExternalOutput")

    with tile.TileContext(nc) as tc:
        with (
            tc.tile_pool(name="const", bufs=1) as constp,
            tc.tile_pool(name="wihp", bufs=24) as wihp,
            tc.tile_pool(name="hp", bufs=2) as hp,
            tc.tile_pool(name="cp", bufs=2) as cp,
            tc.tile_pool(name="prep", bufs=12) as prep,
            tc.tile_pool(name="gp", bufs=10) as gp,
            tc.tile_pool(name="tp", bufs=8) as tp,
            tc.tile_pool(name="psum", bufs=8, space="PSUM") as psump,
        ):
            whh_sb = constp.tile([128, NKH * G4], F16, tag="whh")
            xt_sb = constp.tile([128, NKX * BL], F16, tag="xt")
            xg_sb = constp.tile([128, NM * BL], F16, tag="xg")
            bias_sb = constp.tile([128, NM], F32, tag="bias")

            # ---- input DMAs, ordered by first use: W_ih feeds the xg phase
            # immediately; W_hh is only needed once the xg phase ends.
            # x^T chunk 0 goes alone so the first matmul's inputs land fast.
            wih_tiles = {}

            def load_wih(mg, kc):
                wt = wihp.tile([128, 1024], F16, tag="wih", name="wt")
                nc.sync.dma_start(wt[:, :], wih[mg, kc])
                wih_tiles[(mg, kc)] = wt

            src = xt[:, 0:1, :].rearrange("p k c -> p (k c)")
            nc.sync.dma_start(xt_sb[:, 0:BL], src)
            load_wih(0, 0)
            for q0, q1 in ((1, 4), (4, 10), (10, 16)):
                src = xt[:, q0:q1, :].rearrange("p k c -> p (k c)")
                nc.sync.dma_start(xt_sb[:, q0 * BL:q1 * BL], src)
            for mg in range(NMG):
                if mg == NMG - 1:  # tiny; needed at step 0's first gates
                    nc.sync.dma_start(bias_sb[:, :], bias[:, :])
                for kc in range(NKX):
                    if (mg, kc) == (0, 0):
                        continue
                    load_wih(mg, kc)
            for mg in range(NMG):
                src = whh[mg].rearrange("p k c -> p (k c)")
                nc.sync.dma_start(
                    whh_sb[:, mg * 8192:(mg + 1) * 8192], src)

            # ---- initial state ----
            h_prev = hp.tile([128, NKH * BL], F16, tag="h")
            c_prev = cp.tile([128, NKH * BL], F32, tag="c")
            nc.vector.memset(h_prev[:, :], INIT)
            nc.gpsimd.memset(c_prev[:, :], INIT)

            def whh_col(kc, m):
                mg, ml = m // 8, m % 8
                off = mg * 8192 + kc * 1024 + ml * 128
                return whh_sb[:, off:off + 128]

            # Gate order within an hc-group: g first (t1 = f*c only needs f;
            # t0 = i*g needs i and g), o last (only consumed by the final h
            # mul). PSUM stops then arrive staggered through the group's
            # window and the drain chain overlaps the matmuls.
            GATE_ORDER = (2, 1, 0, 3)

            def rec_matmuls(hc, h_in):
                """Matmuls accumulating the 4 gate m-tiles of h-chunk hc.
                hc 0 runs kc-major with kc 7 last because h[7] of the
                previous step lands just after the step boundary; all other
                groups run gi-major so each gate's accumulator completes (and
                drains) as early as possible."""
                ps = [psump.tile([128, BL], F32, tag="ps", name="ps")
                      for _ in range(4)]
                if hc == 0:
                    for kc in range(NKH):
                        for gi in GATE_ORDER:
                            nc.tensor.matmul(
                                ps[gi][:, :], whh_col(kc, 4 * hc + gi),
                                h_in[:, kc * BL:(kc + 1) * BL],
                                start=(kc == 0), stop=(kc == NKH - 1))
                else:
                    for gi in GATE_ORDER:
                        for kc in range(NKH):
                            nc.tensor.matmul(
                                ps[gi][:, :], whh_col(kc, 4 * hc + gi),
                                h_in[:, kc * BL:(kc + 1) * BL],
                                start=(kc == 0), stop=(kc == NKH - 1))
                return ps

            def drain_hc(t, hc, ps, h_new, c_new):
                """DVE/ACT/DMA ops turning h-chunk hc's 4 PSUM accumulators
                into h/c chunk hc of step t."""
                gates = {}
                for gi in GATE_ORDER:
                    m = 4 * hc + gi
                    pre = prep.tile([128, BL], F32, tag="pre")
                    nc.vector.tensor_add(
                        pre[:, :], ps[gi][:, :], xg_sb[:, m * BL:(m + 1) * BL])
                    g = gp.tile([128, BL], F16, tag="g", name=f"g{gi}")
                    fn = AF.Tanh if gi == 2 else AF.Sigmoid
                    nc.scalar.activation(g[:, :], pre[:, :], fn,
                                         bias=bias_sb[:, m:m + 1])
                    gates[gi] = g
                sl = slice(hc * BL, (hc + 1) * BL)
                t0 = tp.tile([128, BL], F16, tag="t0")
                t1 = tp.tile([128, BL], F32, tag="t1")
                nc.vector.tensor_mul(t1[:, :], gates[1][:, :], c_prev[:, sl])
                nc.vector.tensor_mul(t0[:, :], gates[0][:, :], gates[2][:, :])
                nc.vector.tensor_add(c_new[:, sl], t0[:, :], t1[:, :])
                th = tp.tile([128, BL], F16, tag="th")
                nc.scalar.activation(th[:, :], c_new[:, sl], AF.Tanh)
                nc.vector.tensor_mul(h_new[:, sl], gates[3][:, :], th[:, :])
                nc.sync.dma_start(hs[t, hc], h_new[:, sl])

            # ---- xg phase: xg = W_ih^T x, streamed against the W_ih DMAs.
            # W_hh arrives during this phase and its tail overlaps step 0.
            for hc in range(NKH):
                mg = hc // 2
                mlo = 4 * (hc % 2)  # 0 or 4: this hc's cols in the wih tiles
                psx = [psump.tile([128, BL], F32, tag="ps", name="psx")
                       for _ in range(4)]
                for kc in range(NKX):
                    wt = wih_tiles[(mg, kc)]
                    for gi in range(4):
                        nc.tensor.matmul(
                            psx[gi][:, :],
                            wt[:, (mlo + gi) * 128:(mlo + gi + 1) * 128],
                            xt_sb[:, kc * BL:(kc + 1) * BL],
                            start=(kc == 0), stop=(kc == NKX - 1))
                for gi in range(4):  # xg to SBUF (f16)
                    m = 4 * hc + gi
                    nc.scalar.copy(xg_sb[:, m * BL:(m + 1) * BL],
                                   psx[gi][:, :])

            # ---- steps 0..T-1 ----
            for t in range(T):
                h_new = hp.tile([128, NKH * BL], F16, tag="h")
                c_new = cp.tile([128, NKH * BL], F32, tag="c")
                for hc in range(NKH):
                    ps = rec_matmuls(hc, h_prev)
                    drain_hc(t, hc, ps, h_new, c_new)
                h_prev, c_prev = h_new, c_new

    nc.compile()
    return nc


def timeline_ns():
    from concourse.timeline_sim import TimelineSim
    nc = _get_nc()
    ts = TimelineSim(nc)
    ts.simulate()
    return ts.time


def _get_nc():
    global _cached_nc
    if _cached_nc is None:
        _cached_nc = _build()
    return _cached_nc


def _perm():
    """Gate-row permutation: new position m*128+rr (m = hc*4+gi) <- original
    gate row gi*1024 + hc*128 + rr."""
    gi, hc, rr = np.meshgrid(np.arange(4), np.arange(NKH), np.arange(128),
                             indexing="ij")
    p = np.empty(G4, np.int64)
    m = hc * 4 + gi
    p[(m * 128 + rr).ravel()] = (gi * 1024 + hc * 128 + rr).ravel()
    return p


def make_inputs(x, W_ih, W_hh, b_ih, b_hh):
    """Host-side packing shared by kernel() and the quick tester."""
    f16 = np.float16
    perm = _perm()
    # W_ih^T cols permuted -> [16 kc, 128, 4 mg, 1024] -> [4, 16, 128, 1024]
    wihP = np.ascontiguousarray(
        W_ih.T[:, perm].reshape(NKX, 128, NMG, 1024).transpose(2, 0, 1, 3)
    ).astype(f16)
    whhP = np.ascontiguousarray(
        W_hh.T[:, perm].reshape(NKH, 128, NMG, 1024).transpose(2, 1, 0, 3)
    ).astype(f16)
    biasP = np.ascontiguousarray(
        (b_ih + b_hh)[perm].reshape(NM, 128).T).astype(np.float32)
    in_maps = []
    for c in range(NCORES):
        xtP = np.ascontiguousarray(
            x[c * BL:(c + 1) * BL].T.reshape(NKX, 128, BL).transpose(1, 0, 2)
        ).astype(f16)
        in_maps.append({"wih": wihP, "whh": whhP, "xt": xtP, "bias": biasP})
    return in_maps


def unpack_out(hs_f16):
    """[T, 8, 128, BL] f16 -> [T, BL, H] f32 for one core."""
    return hs_f16.transpose(0, 3, 1, 2).reshape(T, BL, H).astype(np.float32)


def kernel(x, W_ih, W_hh, b_ih, b_hh):
    global LAST_EXEC_NS, LAST_RESULTS
    nc = _get_nc()
    x = np.asarray(x, np.float32)
    in_maps = make_inputs(x, np.asarray(W_ih, np.float32),
                          np.asarray(W_hh, np.float32),
                          np.asarray(b_ih, np.float32),
                          np.asarray(b_hh, np.float32))
    trace = os.environ.get("LSTM_TRACE") == "1"
    res = run_bass_kernel_spmd(
        nc, in_maps, core_ids=list(range(NCORES)), trace=trace
    )
    LAST_EXEC_NS = res.exec_time_ns
    LAST_RESULTS = res

    out = np.empty((T, B, H), np.float32)
    for c in range(NCORES):
        out[:, c * BL:(c + 1) * BL, :] = unpack_out(res.results[c]["hs"])
    return out


# revision 26
# speedup vs baseline: 1.5639x; 1.0032x over previous
"""LSTM regression kernel for 8 Trainium2 NeuronCores.

Model (reference): B=2048, IN=2048, H=1024, T=15 steps, x constant across
steps. Data-parallel over batch: each of the 8 cores handles 256 batch rows.

Per-core design (BL=256 batch cols, everything kept transposed [rows, BL]):
 - Gate rows are host-permuted hc-major: m-tile m = hc*4 + gi (hc = h-chunk
   0..7, gi = gate i/f/g/o). The 4 gates of h-chunk hc live in 4 consecutive
   m-tiles, so each step is processed as 8 hc-groups of 4 m-tiles; cell
   updates spread evenly across the step and the next step's matmuls (which
   consume h chunks in kc-ascending order) never stall on the previous
   step's tail.
 - xgT[4096, BL] = W_ih^T x computed once at start, single fp16 product
   (hi/lo splitting unnecessary for the 2e-2 error budget), stored f16.
 - Per step: gatesT = W_hh^T h accumulated in PSUM over 8 K-chunks (pure
   8 matmuls per m-tile - no identity-matmul adds). xg is added on the DVE
   (tensor_add reading PSUM), gate bias b_ih+b_hh is folded into the
   ScalarE activation's per-partition bias operand. Cell update on DVE with
   f16 gates (f32 cell state); h is produced directly in f16 for the next
   matmul and DMA'd out per-chunk as f16 (host converts to f32).
 - PSUM: one [128,256] f32 accumulator per bank; hc-group g uses banks
   (4g mod 8)..+3 so group g+1 accumulates while group g drains.
 - Weights are host-packed so every DMA is contiguous with >=2KB
   per-partition rows; W_ih streams per k-chunk overlapped with the xg
   matmuls, W_hh per m-group between them.
"""

import os
import numpy as np
import ml_dtypes

try:
    import concourse.bass as bass
except ImportError:  # pragma: no cover
    import sys
    sys.path.insert(0, "/opt/trn_rl_repo")
    import concourse.bass as bass
from concourse import bacc
import concourse.mybir as mybir
import concourse.tile as tile
from concourse.bass_utils import run_bass_kernel_spmd

F32 = mybir.dt.float32
F16 = mybir.dt.float16
AF = mybir.ActivationFunctionType

T = 15
B, IN, H = 2048, 2048, 1024
NCORES = 8
BL = B // NCORES            # 256 batch rows per core
G4 = 4 * H                  # 4096 gate rows
NM = G4 // 128              # 32 gate m-tiles
NMG = 4                     # m-groups of 8 m-tiles (W DMA granularity)
NKH = H // 128              # 8 hidden K-chunks
NKX = IN // 128             # 16 input K-chunks
INIT = 0.01

LAST_EXEC_NS = None
LAST_RESULTS = None

_cached_nc = None


def _build():
    nc = bacc.Bacc(None, target_bir_lowering=False)
    # [mg][kc][128][1024]: W_ih^T k-chunk rows x this m-group's 8*128 cols
    wih = nc.dram_tensor("wih", [NMG, NKX, 128, 1024], F16, kind="ExternalInput")
    # [mg][128][kc][1024]: W_hh^T, partition-major so the per-mg DMA groups
    # (k c) contiguously per partition row
    whh = nc.dram_tensor("whh", [NMG, 128, NKH, 1024], F16, kind="ExternalInput")
    # x^T partition-major: [128][kc][BL]
    xt = nc.dram_tensor("xt", [128, NKX, BL], F16, kind="ExternalInput")
    bias = nc.dram_tensor("bias", [128, NM], F32, kind="ExternalInput")
    hs = nc.dram_tensor("hs", [T, NKH, 128, BL], F16, kind="ExternalOutput")

    with tile.TileContext(nc) as tc:
        with (
            tc.tile_pool(name="const", bufs=1) as constp,
            tc.tile_pool(name="wihp", bufs=24) as wihp,
            tc.tile_pool(name="hp", bufs=2) as hp,
            tc.tile_pool(name="cp", bufs=2) as cp,
            tc.tile_pool(name="prep", bufs=12) as prep,
            tc.tile_pool(name="gp", bufs=10) as gp,
            tc.tile_pool(name="tp", bufs=8) as tp,
            tc.tile_pool(name="psum", bufs=8, space="PSUM") as psump,
        ):
            whh_sb = constp.tile([128, NKH * G4], F16, tag="whh")
            xt_sb = constp.tile([128, NKX * BL], F16, tag="xt")
            xg_sb = constp.tile([128, NM * BL], F16, tag="xg")
            bias_sb = constp.tile([128, NM], F32, tag="bias")

            # ---- input DMAs, ordered by first use: W_ih feeds the xg phase
            # immediately; W_hh is only needed once the xg phase ends.
            # x^T chunk 0 goes alone so the first matmul's inputs land fast.
            wih_tiles = {}

            def load_wih(mg, kc):
                wt = wihp.tile([128, 1024], F16, tag="wih", name="wt")
                nc.sync.dma_start(wt[:, :], wih[mg, kc])
                wih_tiles[(mg, kc)] = wt

            load_wih(0, 0)
            src = xt[:, 0:1, :].rearrange("p k c -> p (k c)")
            nc.sync.dma_start(xt_sb[:, 0:BL], src)
            for q0, q1 in ((1, 4), (4, 10), (10, 16)):
                src = xt[:, q0:q1, :].rearrange("p k c -> p (k c)")
                nc.sync.dma_start(xt_sb[:, q0 * BL:q1 * BL], src)
            for mg in range(NMG):
                if mg == NMG - 1:  # tiny; needed at step 0's first gates
                    nc.sync.dma_start(bias_sb[:, :], bias[:, :])
                for kc in range(NKX):
                    if (mg, kc) == (0, 0):
                        continue
                    load_wih(mg, kc)
            for mg in range(NMG):
                src = whh[mg].rearrange("p k c -> p (k c)")
                nc.sync.dma_start(
                    whh_sb[:, mg * 8192:(mg + 1) * 8192], src)

            # ---- initial state ----
            h_prev = hp.tile([128, NKH * BL], F16, tag="h")
            c_prev = cp.tile([128, NKH * BL], F32, tag="c")
            nc.vector.memset(h_prev[:, :], INIT)
            nc.gpsimd.memset(c_prev[:, :], INIT)

            def whh_col(kc, m):
                mg, ml = m // 8, m % 8
                off = mg * 8192 + kc * 1024 + ml * 128
                return whh_sb[:, off:off + 128]

            # Gate order within an hc-group: g first (t1 = f*c only needs f;
            # t0 = i*g needs i and g), o last (only consumed by the final h
            # mul). PSUM stops then arrive staggered through the group's
            # window and the drain chain overlaps the matmuls.
            GATE_ORDER = (2, 1, 0, 3)

            def rec_matmuls(hc, h_in):
                """Matmuls accumulating the 4 gate m-tiles of h-chunk hc.
                hc 0 runs kc-major with kc 7 last because h[7] of the
                previous step lands just after the step boundary; all other
                groups run gi-major so each gate's accumulator completes (and
                drains) as early as possible."""
                ps = [psump.tile([128, BL], F32, tag="ps", name="ps")
                      for _ in range(4)]
                if hc == 0:
                    for kc in range(NKH):
                        for gi in GATE_ORDER:
                            nc.tensor.matmul(
                                ps[gi][:, :], whh_col(kc, 4 * hc + gi),
                                h_in[:, kc * BL:(kc + 1) * BL],
                                start=(kc == 0), stop=(kc == NKH - 1))
                else:
                    for gi in GATE_ORDER:
                        for kc in range(NKH):
                            nc.tensor.matmul(
                                ps[gi][:, :], whh_col(kc, 4 * hc + gi),
                                h_in[:, kc * BL:(kc + 1) * BL],
                                start=(kc == 0), stop=(kc == NKH - 1))
                return ps

            def drain_hc(t, hc, ps, h_new, c_new):
                """DVE/ACT/DMA ops turning h-chunk hc's 4 PSUM accumulators
                into h/c chunk hc of step t."""
                gates = {}
                for gi in GATE_ORDER:
                    m = 4 * hc + gi
                    pre = prep.tile([128, BL], F32, tag="pre")
                    nc.vector.tensor_add(
                        pre[:, :], ps[gi][:, :], xg_sb[:, m * BL:(m + 1) * BL])
                    g = gp.tile([128, BL], F16, tag="g", name=f"g{gi}")
                    fn = AF.Tanh if gi == 2 else AF.Sigmoid
                    nc.scalar.activation(g[:, :], pre[:, :], fn,
                                         bias=bias_sb[:, m:m + 1])
                    gates[gi] = g
                sl = slice(hc * BL, (hc + 1) * BL)
                t0 = tp.tile([128, BL], F16, tag="t0")
                t1 = tp.tile([128, BL], F32, tag="t1")
                nc.vector.tensor_mul(t1[:, :], gates[1][:, :], c_prev[:, sl])
                nc.vector.tensor_mul(t0[:, :], gates[0][:, :], gates[2][:, :])
                nc.vector.tensor_add(c_new[:, sl], t0[:, :], t1[:, :])
                th = tp.tile([128, BL], F16, tag="th")
                nc.scalar.activation(th[:, :], c_new[:, sl], AF.Tanh)
                nc.vector.tensor_mul(h_new[:, sl], gates[3][:, :], th[:, :])
                nc.sync.dma_start(hs[t, hc], h_new[:, sl])

            # ---- xg phase: xg = W_ih^T x, streamed against the W_ih DMAs.
            # W_hh arrives during this phase and its tail overlaps step 0.
            for hc in range(NKH):
                mg = hc // 2
                mlo = 4 * (hc % 2)  # 0 or 4: this hc's cols in the wih tiles
                psx = [psump.tile([128, BL], F32, tag="ps", name="psx")
                       for _ in range(4)]
                for kc in range(NKX):
                    wt = wih_tiles[(mg, kc)]
                    for gi in range(4):
                        nc.tensor.matmul(
                            psx[gi][:, :],
                            wt[:, (mlo + gi) * 128:(mlo + gi + 1) * 128],
                            xt_sb[:, kc * BL:(kc + 1) * BL],
                            start=(kc == 0), stop=(kc == NKX - 1))
                # xg to SBUF (f16) on the DVE: the ACT engine must enter
                # step 0 without a copy backlog or its drains lag the PE
                for gi in range(4):
                    m = 4 * hc + gi
                    nc.vector.tensor_copy(xg_sb[:, m * BL:(m + 1) * BL],
                                          psx[gi][:, :])

            # ---- steps 0..T-1 ----
            for t in range(T):
                h_new = hp.tile([128, NKH * BL], F16, tag="h")
                c_new = cp.tile([128, NKH * BL], F32, tag="c")
                for hc in range(NKH):
                    ps = rec_matmuls(hc, h_prev)
                    drain_hc(t, hc, ps, h_new, c_new)
                h_prev, c_prev = h_new, c_new

    nc.compile()
    return nc


def timeline_ns():
    from concourse.timeline_sim import TimelineSim
    nc = _get_nc()
    ts = TimelineSim(nc)
    ts.simulate()
    return ts.time


def _get_nc():
    global _cached_nc
    if _cached_nc is None:
        _cached_nc = _build()
    return _cached_nc


def _perm():
    """Gate-row permutation: new position m*128+rr (m = hc*4+gi) <- original
    gate row gi*1024 + hc*128 + rr."""
    gi, hc, rr = np.meshgrid(np.arange(4), np.arange(NKH), np.arange(128),
                             indexing="ij")
    p = np.empty(G4, np.int64)
    m = hc * 4 + gi
    p[(m * 128 + rr).ravel()] = (gi * 1024 + hc * 128 + rr).ravel()
    return p


def make_inputs(x, W_ih, W_hh, b_ih, b_hh):
    """Host-side packing shared by kernel() and the quick tester."""
    f16 = np.float16
    perm = _perm()
    # W_ih^T cols permuted -> [16 kc, 128, 4 mg, 1024] -> [4, 16, 128, 1024]
    wihP = np.ascontiguousarray(
        W_ih.T[:, perm].reshape(NKX, 128, NMG, 1024).transpose(2, 0, 1, 3)
    ).astype(f16)
    whhP = np.ascontiguousarray(
        W_hh.T[:, perm].reshape(NKH, 128, NMG, 1024).transpose(2, 1, 0, 3)
    ).astype(f16)
    biasP = np.ascontiguousarray(
        (b_ih + b_hh)[perm].reshape(NM, 128).T).astype(np.float32)
    in_maps = []
    for c in range(NCORES):
        xtP = np.ascontiguousarray(
            x[c * BL:(c + 1) * BL].T.reshape(NKX, 128, BL).transpose(1, 0, 2)
        ).astype(f16)
        in_maps.append({"wih": wihP, "whh": whhP, "xt": xtP, "bias": biasP})
    return in_maps


def unpack_out(hs_f16):
    """[T, 8, 128, BL] f16 -> [T, BL, H] f32 for one core."""
    return hs_f16.transpose(0, 3, 1, 2).reshape(T, BL, H).astype(np.float32)


def kernel(x, W_ih, W_hh, b_ih, b_hh):
    global LAST_EXEC_NS, LAST_RESULTS
    nc = _get_nc()
    x = np.asarray(x, np.float32)
    in_maps = make_inputs(x, np.asarray(W_ih, np.float32),
                          np.asarray(W_hh, np.float32),
                          np.asarray(b_ih, np.float32),
                          np.asarray(b_hh, np.float32))
    trace = os.environ.get("LSTM_TRACE") == "1"
    res = run_bass_kernel_spmd(
        nc, in_maps, core_ids=list(range(NCORES)), trace=trace
    )
    LAST_EXEC_NS = res.exec_time_ns
    LAST_RESULTS = res

    out = np.empty((T, B, H), np.float32)
    for c in range(NCORES):
        out[:, c * BL:(c + 1) * BL, :] = unpack_out(res.results[c]["hs"])
    return out


# revision 27
# speedup vs baseline: 1.5650x; 1.0007x over previous
"""LSTM regression kernel for 8 Trainium2 NeuronCores.

Model (reference): B=2048, IN=2048, H=1024, T=15 steps, x constant across
steps. Data-parallel over batch: each of the 8 cores handles 256 batch rows.

Per-core design (BL=256 batch cols, everything kept transposed [rows, BL]):
 - Gate rows are host-permuted hc-major: m-tile m = hc*4 + gi (hc = h-chunk
   0..7, gi = gate i/f/g/o). The 4 gates of h-chunk hc live in 4 consecutive
   m-tiles, so each step is processed as 8 hc-groups of 4 m-tiles; cell
   updates spread evenly across the step and the next step's matmuls (which
   consume h chunks in kc-ascending order) never stall on the previous
   step's tail.
 - xgT[4096, BL] = W_ih^T x computed once at start, single fp16 product
   (hi/lo splitting unnecessary for the 2e-2 error budget), stored f16.
 - Per step: gatesT = W_hh^T h accumulated in PSUM over 8 K-chunks (pure
   8 matmuls per m-tile - no identity-matmul adds). xg is added on the DVE
   (tensor_add reading PSUM), gate bias b_ih+b_hh is folded into the
   ScalarE activation's per-partition bias operand. Cell update on DVE with
   f16 gates (f32 cell state); h is produced directly in f16 for the next
   matmul and DMA'd out per-chunk as f16 (host converts to f32).
 - PSUM: one [128,256] f32 accumulator per bank; hc-group g uses banks
   (4g mod 8)..+3 so group g+1 accumulates while group g drains.
 - Weights are host-packed so every DMA is contiguous with >=2KB
   per-partition rows; W_ih streams per k-chunk overlapped with the xg
   matmuls, W_hh per m-group between them.
"""

import os
import numpy as np
import ml_dtypes

try:
    import concourse.bass as bass
except ImportError:  # pragma: no cover
    import sys
    sys.path.insert(0, "/opt/trn_rl_repo")
    import concourse.bass as bass
from concourse import bacc
import concourse.mybir as mybir
import concourse.tile as tile
from concourse.bass_utils import run_bass_kernel_spmd

F32 = mybir.dt.float32
F16 = mybir.dt.float16
AF = mybir.ActivationFunctionType

T = 15
B, IN, H = 2048, 2048, 1024
NCORES = 8
BL = B // NCORES            # 256 batch rows per core
G4 = 4 * H                  # 4096 gate rows
NM = G4 // 128              # 32 gate m-tiles
NMG = 4                     # m-groups of 8 m-tiles (W DMA granularity)
NKH = H // 128              # 8 hidden K-chunks
NKX = IN // 128             # 16 input K-chunks
INIT = 0.01

LAST_EXEC_NS = None
LAST_RESULTS = None

_cached_nc = None


def _build():
    nc = bacc.Bacc(None, target_bir_lowering=False)
    # [mg][kc][128][1024]: W_ih^T k-chunk rows x this m-group's 8*128 cols
    wih = nc.dram_tensor("wih", [NMG, NKX, 128, 1024], F16, kind="ExternalInput")
    # [mg][128][kc][1024]: W_hh^T, partition-major so the per-mg DMA groups
    # (k c) contiguously per partition row
    whh = nc.dram_tensor("whh", [NMG, 128, NKH, 1024], F16, kind="ExternalInput")
    # x^T partition-major: [128][kc][BL]
    xt = nc.dram_tensor("xt", [128, NKX, BL], F16, kind="ExternalInput")
    bias = nc.dram_tensor("bias", [128, NM], F32, kind="ExternalInput")
    hs = nc.dram_tensor("hs", [T, NKH, 128, BL], F16, kind="ExternalOutput")

    with tile.TileContext(nc) as tc:
        with (
            tc.tile_pool(name="const", bufs=1) as constp,
            tc.tile_pool(name="wihp", bufs=24) as wihp,
            tc.tile_pool(name="hp", bufs=2) as hp,
            tc.tile_pool(name="cp", bufs=2) as cp,
            tc.tile_pool(name="prep", bufs=12) as prep,
            tc.tile_pool(name="gp", bufs=10) as gp,
            tc.tile_pool(name="tp", bufs=8) as tp,
            tc.tile_pool(name="psum", bufs=8, space="PSUM") as psump,
        ):
            whh_sb = constp.tile([128, NKH * G4], F16, tag="whh")
            xt_sb = constp.tile([128, NKX * BL], F16, tag="xt")
            xg_sb = constp.tile([128, NM * BL], F16, tag="xg")
            bias_sb = constp.tile([128, NM], F32, tag="bias")

            # ---- input DMAs, ordered by first use: W_ih feeds the xg phase
            # immediately; W_hh is only needed once the xg phase ends.
            # x^T chunk 0 goes alone so the first matmul's inputs land fast.
            wih_tiles = {}

            def load_wih(mg, kc):
                wt = wihp.tile([128, 1024], F16, tag="wih", name="wt")
                nc.sync.dma_start(wt[:, :], wih[mg, kc])
                wih_tiles[(mg, kc)] = wt

            load_wih(0, 0)
            # x^T chunks interleaved with the first W_ih tiles so neither
            # starves the PE while it ramps up
            for (q0, q1), kc in (((0, 1), 1), ((1, 4), 2), ((4, 10), 3),
                                 ((10, 16), 4)):
                src = xt[:, q0:q1, :].rearrange("p k c -> p (k c)")
                nc.sync.dma_start(xt_sb[:, q0 * BL:q1 * BL], src)
                load_wih(0, kc)
            for mg in range(NMG):
                if mg == NMG - 1:  # tiny; needed at step 0's first gates
                    nc.sync.dma_start(bias_sb[:, :], bias[:, :])
                for kc in range(NKX):
                    if (mg, kc) in wih_tiles:
                        continue
                    load_wih(mg, kc)
            for mg in range(NMG):
                src = whh[mg].rearrange("p k c -> p (k c)")
                nc.sync.dma_start(
                    whh_sb[:, mg * 8192:(mg + 1) * 8192], src)

            # ---- initial state ----
            h_prev = hp.tile([128, NKH * BL], F16, tag="h")
            c_prev = cp.tile([128, NKH * BL], F32, tag="c")
            nc.vector.memset(h_prev[:, :], INIT)
            nc.gpsimd.memset(c_prev[:, :], INIT)

            def whh_col(kc, m):
                mg, ml = m // 8, m % 8
                off = mg * 8192 + kc * 1024 + ml * 128
                return whh_sb[:, off:off + 128]

            # Gate order within an hc-group: g first (t1 = f*c only needs f;
            # t0 = i*g needs i and g), o last (only consumed by the final h
            # mul). PSUM stops then arrive staggered through the group's
            # window and the drain chain overlaps the matmuls.
            GATE_ORDER = (2, 1, 0, 3)

            def rec_matmuls(hc, h_in):
                """Matmuls accumulating the 4 gate m-tiles of h-chunk hc.
                hc 0 runs kc-major with kc 7 last because h[7] of the
                previous step lands just after the step boundary; all other
                groups run gi-major so each gate's accumulator completes (and
                drains) as early as possible."""
                ps = [psump.tile([128, BL], F32, tag="ps", name="ps")
                      for _ in range(4)]
                if hc == 0:
                    for kc in range(NKH):
                        for gi in GATE_ORDER:
                            nc.tensor.matmul(
                                ps[gi][:, :], whh_col(kc, 4 * hc + gi),
                                h_in[:, kc * BL:(kc + 1) * BL],
                                start=(kc == 0), stop=(kc == NKH - 1))
                else:
                    for gi in GATE_ORDER:
                        for kc in range(NKH):
                            nc.tensor.matmul(
                                ps[gi][:, :], whh_col(kc, 4 * hc + gi),
                                h_in[:, kc * BL:(kc + 1) * BL],
                                start=(kc == 0), stop=(kc == NKH - 1))
                return ps

            def drain_hc(t, hc, ps, h_new, c_new):
                """DVE/ACT/DMA ops turning h-chunk hc's 4 PSUM accumulators
                into h/c chunk hc of step t."""
                gates = {}
                for gi in GATE_ORDER:
                    m = 4 * hc + gi
                    pre = prep.tile([128, BL], F32, tag="pre")
                    nc.vector.tensor_add(
                        pre[:, :], ps[gi][:, :], xg_sb[:, m * BL:(m + 1) * BL])
                    g = gp.tile([128, BL], F16, tag="g", name=f"g{gi}")
                    fn = AF.Tanh if gi == 2 else AF.Sigmoid
                    nc.scalar.activation(g[:, :], pre[:, :], fn,
                                         bias=bias_sb[:, m:m + 1])
                    gates[gi] = g
                sl = slice(hc * BL, (hc + 1) * BL)
                t0 = tp.tile([128, BL], F16, tag="t0")
                t1 = tp.tile([128, BL], F32, tag="t1")
                nc.vector.tensor_mul(t1[:, :], gates[1][:, :], c_prev[:, sl])
                nc.vector.tensor_mul(t0[:, :], gates[0][:, :], gates[2][:, :])
                nc.vector.tensor_add(c_new[:, sl], t0[:, :], t1[:, :])
                th = tp.tile([128, BL], F16, tag="th")
                nc.scalar.activation(th[:, :], c_new[:, sl], AF.Tanh)
                nc.vector.tensor_mul(h_new[:, sl], gates[3][:, :], th[:, :])
                nc.sync.dma_start(hs[t, hc], h_new[:, sl])

            # ---- xg phase: xg = W_ih^T x, streamed against the W_ih DMAs.
            # W_hh arrives during this phase and its tail overlaps step 0.
            for hc in range(NKH):
                mg = hc // 2
                mlo = 4 * (hc % 2)  # 0 or 4: this hc's cols in the wih tiles
                psx = [psump.tile([128, BL], F32, tag="ps", name="psx")
                       for _ in range(4)]
                for kc in range(NKX):
                    wt = wih_tiles[(mg, kc)]
                    for gi in range(4):
                        nc.tensor.matmul(
                            psx[gi][:, :],
                            wt[:, (mlo + gi) * 128:(mlo + gi + 1) * 128],
                            xt_sb[:, kc * BL:(kc + 1) * BL],
                            start=(kc == 0), stop=(kc == NKX - 1))
                # xg to SBUF (f16) on the DVE: the ACT engine must enter
                # step 0 without a copy backlog or its drains lag the PE
                for gi in range(4):
                    m = 4 * hc + gi
                    nc.vector.tensor_copy(xg_sb[:, m * BL:(m + 1) * BL],
                                          psx[gi][:, :])

            # ---- steps 0..T-1 ----
            for t in range(T):
                h_new = hp.tile([128, NKH * BL], F16, tag="h")
                c_new = cp.tile([128, NKH * BL], F32, tag="c")
                for hc in range(NKH):
                    ps = rec_matmuls(hc, h_prev)
                    drain_hc(t, hc, ps, h_new, c_new)
                h_prev, c_prev = h_new, c_new

    nc.compile()
    return nc


def timeline_ns():
    from concourse.timeline_sim import TimelineSim
    nc = _get_nc()
    ts = TimelineSim(nc)
    ts.simulate()
    return ts.time


def _get_nc():
    global _cached_nc
    if _cached_nc is None:
        _cached_nc = _build()
    return _cached_nc


def _perm():
    """Gate-row permutation: new position m*128+rr (m = hc*4+gi) <- original
    gate row gi*1024 + hc*128 + rr."""
    gi, hc, rr = np.meshgrid(np.arange(4), np.arange(NKH), np.arange(128),
                             indexing="ij")
    p = np.empty(G4, np.int64)
    m = hc * 4 + gi
    p[(m * 128 + rr).ravel()] = (gi * 1024 + hc * 128 + rr).ravel()
    return p


def make_inputs(x, W_ih, W_hh, b_ih, b_hh):
    """Host-side packing shared by kernel() and the quick tester."""
    f16 = np.float16
    perm = _perm()
    # W_ih^T cols permuted -> [16 kc, 128, 4 mg, 1024] -> [4, 16, 128, 1024]
    wihP = np.ascontiguousarray(
        W_ih.T[:, perm].reshape(NKX, 128, NMG, 1024).transpose(2, 0, 1, 3)
    ).astype(f16)
    whhP = np.ascontiguousarray(
        W_hh.T[:, perm].reshape(NKH, 128, NMG, 1024).transpose(2, 1, 0, 3)
    ).astype(f16)
    biasP = np.ascontiguousarray(
        (b_ih + b_hh)[perm].reshape(NM, 128).T).astype(np.float32)
    in_maps = []
    for c in range(NCORES):
        xtP = np.ascontiguousarray(
            x[c * BL:(c + 1) * BL].T.reshape(NKX, 128, BL).transpose(1, 0, 2)
        ).astype(f16)
        in_maps.append({"wih": wihP, "whh": whhP, "xt": xtP, "bias": biasP})
    return in_maps


def unpack_out(hs_f16):
    """[T, 8, 128, BL] f16 -> [T, BL, H] f32 for one core."""
    return hs_f16.transpose(0, 3, 1, 2).reshape(T, BL, H).astype(np.float32)


def kernel(x, W_ih, W_hh, b_ih, b_hh):
    global LAST_EXEC_NS, LAST_RESULTS
    nc = _get_nc()
    x = np.asarray(x, np.float32)
    in_maps = make_inputs(x, np.asarray(W_ih, np.float32),
                          np.asarray(W_hh, np.float32),
                          np.asarray(b_ih, np.float32),
                          np.asarray(b_hh, np.float32))
    trace = os.environ.get("LSTM_TRACE") == "1"
    res = run_bass_kernel_spmd(
        nc, in_maps, core_ids=list(range(NCORES)), trace=trace
    )
    LAST_EXEC_NS = res.exec_time_ns
    LAST_RESULTS = res

    out = np.empty((T, B, H), np.float32)
    for c in range(NCORES):
        out[:, c * BL:(c + 1) * BL, :] = unpack_out(res.results[c]["hs"])
    return out


# revision 34
# speedup vs baseline: 1.5692x; 1.0027x over previous
"""LSTM regression kernel for 8 Trainium2 NeuronCores.

Model (reference): B=2048, IN=2048, H=1024, T=15 steps, x constant across
steps. Data-parallel over batch: each of the 8 cores handles 256 batch rows.

Per-core design (BL=256 batch cols, everything kept transposed [rows, BL]):
 - Gate rows are host-permuted hc-major: m-tile m = hc*4 + gi (hc = h-chunk
   0..7, gi = gate i/f/g/o). The 4 gates of h-chunk hc live in 4 consecutive
   m-tiles, so each step is processed as 8 hc-groups of 4 m-tiles; cell
   updates spread evenly across the step and the next step's matmuls (which
   consume h chunks in kc-ascending order) never stall on the previous
   step's tail.
 - xgT[4096, BL] = W_ih^T x computed once at start, single fp16 product
   (hi/lo splitting unnecessary for the 2e-2 error budget), stored f16.
 - Per step: gatesT = W_hh^T h accumulated in PSUM over 8 K-chunks (pure
   8 matmuls per m-tile - no identity-matmul adds). xg is added on the DVE
   (tensor_add reading PSUM), gate bias b_ih+b_hh is folded into the
   ScalarE activation's per-partition bias operand. Cell update on DVE with
   f16 gates (f32 cell state); h is produced directly in f16 for the next
   matmul and DMA'd out per-chunk as f16 (host converts to f32).
 - PSUM: one [128,256] f32 accumulator per bank; hc-group g uses banks
   (4g mod 8)..+3 so group g+1 accumulates while group g drains.
 - Weights are host-packed so every DMA is contiguous with >=2KB
   per-partition rows; W_ih streams per k-chunk overlapped with the xg
   matmuls, W_hh per m-group between them.
"""

import os
import numpy as np
import ml_dtypes

try:
    import concourse.bass as bass
except ImportError:  # pragma: no cover
    import sys
    sys.path.insert(0, "/opt/trn_rl_repo")
    import concourse.bass as bass
from concourse import bacc
import concourse.mybir as mybir
import concourse.tile as tile
from concourse.bass_utils import run_bass_kernel_spmd
from concourse.masks import make_identity

F32 = mybir.dt.float32
F16 = mybir.dt.float16
AF = mybir.ActivationFunctionType

T = 15
B, IN, H = 2048, 2048, 1024
NCORES = 8
BL = B // NCORES            # 256 batch rows per core
G4 = 4 * H                  # 4096 gate rows
NM = G4 // 128              # 32 gate m-tiles
NMG = 4                     # m-groups of 8 m-tiles (W DMA granularity)
NKH = H // 128              # 8 hidden K-chunks
NKX = IN // 128             # 16 input K-chunks
INIT = 0.01

LAST_EXEC_NS = None
LAST_RESULTS = None

_cached_nc = None


def _build():
    nc = bacc.Bacc(None, target_bir_lowering=False)
    # [mg][kc][128][1024]: W_ih^T k-chunk rows x this m-group's 8*128 cols
    wih = nc.dram_tensor("wih", [NMG, NKX, 128, 1024], F16, kind="ExternalInput")
    # [mg][128][kc][1024]: W_hh^T, partition-major so the per-mg DMA groups
    # (k c) contiguously per partition row
    whh = nc.dram_tensor("whh", [NMG, 128, NKH, 1024], F16, kind="ExternalInput")
    # x^T partition-major: [128][kc][BL]
    xt = nc.dram_tensor("xt", [128, NKX, BL], F16, kind="ExternalInput")
    bias = nc.dram_tensor("bias", [128, NM], F32, kind="ExternalInput")
    hs = nc.dram_tensor("hs", [T, NKH, 128, BL], F16, kind="ExternalOutput")

    with tile.TileContext(nc) as tc:
        with (
            tc.tile_pool(name="const", bufs=1) as constp,
            tc.tile_pool(name="wihp", bufs=24) as wihp,
            tc.tile_pool(name="hp", bufs=2) as hp,
            tc.tile_pool(name="cp", bufs=2) as cp,
            tc.tile_pool(name="prep", bufs=12) as prep,
            tc.tile_pool(name="gp", bufs=10) as gp,
            tc.tile_pool(name="tp", bufs=8) as tp,
            tc.tile_pool(name="psum", bufs=8, space="PSUM") as psump,
        ):
            whh_sb = constp.tile([128, NKH * G4], F16, tag="whh")
            xt_sb = constp.tile([128, NKX * BL], F16, tag="xt")
            xg_sb = constp.tile([128, NM * BL], F16, tag="xg")
            bias_sb = constp.tile([128, NM], F32, tag="bias")
            ident = constp.tile([128, 128], F16, tag="ident")
            warm = constp.tile([128, 128], F16, tag="warm")

            # ---- input DMAs, ordered by first use: W_ih feeds the xg phase
            # immediately; W_hh is only needed once the xg phase ends.
            # x^T chunk 0 goes alone so the first matmul's inputs land fast.
            wih_tiles = {}

            def load_wih(mg, kc):
                wt = wihp.tile([128, 1024], F16, tag="wih", name="wt")
                nc.sync.dma_start(wt[:, :], wih[mg, kc])
                wih_tiles[(mg, kc)] = wt

            load_wih(0, 0)
            # x^T chunks interleaved with the first W_ih tiles so neither
            # starves the PE while it ramps up
            for (q0, q1), kc in (((0, 1), 1), ((1, 4), 2), ((4, 10), 3),
                                 ((10, 16), 4)):
                src = xt[:, q0:q1, :].rearrange("p k c -> p (k c)")
                nc.sync.dma_start(xt_sb[:, q0 * BL:q1 * BL], src)
                load_wih(0, kc)
            for mg in range(NMG):
                if mg == NMG - 1:  # tiny; needed at step 0's first gates
                    nc.sync.dma_start(bias_sb[:, :], bias[:, :])
                for kc in range(NKX):
                    if (mg, kc) in wih_tiles:
                        continue
                    load_wih(mg, kc)
            for mg in range(NMG):
                src = whh[mg].rearrange("p k c -> p (k c)")
                nc.sync.dma_start(
                    whh_sb[:, mg * 8192:(mg + 1) * 8192], src)

            # ---- initial state ----
            nc.vector.memset(warm[:, :], INIT)
            h_prev = hp.tile([128, NKH * BL], F16, tag="h")
            c_prev = cp.tile([128, NKH * BL], F32, tag="c")
            nc.vector.memset(h_prev[:, :], INIT)
            nc.gpsimd.memset(c_prev[:, :], INIT)
            make_identity(nc, ident[:, :])

            # ---- PE warm-up: the tensor engine needs ~3us of continuous
            # work to reach its top p-state; burn tiny matmuls on a dummy
            # tile while the first W_ih/x DMAs are still in flight ----
            ps_warm = psump.tile([128, BL], F32, tag="ps", name="pswarm")
            for i in range(220):
                nc.tensor.matmul(ps_warm[:, 0:16], warm[:, :], warm[:, 0:16],
                                 start=(i == 0), stop=(i == 219))

            def whh_col(kc, m):
                mg, ml = m // 8, m % 8
                off = mg * 8192 + kc * 1024 + ml * 128
                return whh_sb[:, off:off + 128]

            # Gate order within an hc-group: g first (t1 = f*c only needs f;
            # t0 = i*g needs i and g), o last (only consumed by the final h
            # mul). PSUM stops then arrive staggered through the group's
            # window and the drain chain overlaps the matmuls.
            GATE_ORDER = (2, 1, 0, 3)

            def rec_matmuls(hc, h_in, ident_xg=False):
                """Matmuls accumulating the 4 gate m-tiles of h-chunk hc.
                hc 0 runs kc-major with kc 7 last because h[7] of the
                previous step lands just after the step boundary; all other
                groups run gi-major so each gate's accumulator completes (and
                drains) as early as possible. With ident_xg the xg add is
                done here on the PE (identity matmul per gate) - used for the
                very last chunk so the closing drain chain skips the DVE
                pre-add."""
                ps = [psump.tile([128, BL], F32, tag="ps", name="ps")
                      for _ in range(4)]
                if hc == 0:
                    for kc in range(NKH):
                        for gi in GATE_ORDER:
                            nc.tensor.matmul(
                                ps[gi][:, :], whh_col(kc, 4 * hc + gi),
                                h_in[:, kc * BL:(kc + 1) * BL],
                                start=(kc == 0), stop=(kc == NKH - 1))
                else:
                    for gi in GATE_ORDER:
                        for kc in range(NKH):
                            nc.tensor.matmul(
                                ps[gi][:, :], whh_col(kc, 4 * hc + gi),
                                h_in[:, kc * BL:(kc + 1) * BL],
                                start=(kc == 0),
                                stop=(kc == NKH - 1 and not ident_xg))
                        if ident_xg:
                            m = 4 * hc + gi
                            nc.tensor.matmul(
                                ps[gi][:, :], ident[:, :],
                                xg_sb[:, m * BL:(m + 1) * BL],
                                start=False, stop=True)
                return ps

            def drain_hc(t, hc, ps, h_new, c_new, ident_xg=False):
                """DVE/ACT/DMA ops turning h-chunk hc's 4 PSUM accumulators
                into h/c chunk hc of step t."""
                gates = {}
                for gi in GATE_ORDER:
                    m = 4 * hc + gi
                    if ident_xg:
                        src = ps[gi]
                    else:
                        src = prep.tile([128, BL], F32, tag="pre")
                        nc.vector.tensor_add(
                            src[:, :], ps[gi][:, :],
                            xg_sb[:, m * BL:(m + 1) * BL])
                    g = gp.tile([128, BL], F16, tag="g", name=f"g{gi}")
                    fn = AF.Tanh if gi == 2 else AF.Sigmoid
                    nc.scalar.activation(g[:, :], src[:, :], fn,
                                         bias=bias_sb[:, m:m + 1])
                    gates[gi] = g
                sl = slice(hc * BL, (hc + 1) * BL)
                t0 = tp.tile([128, BL], F16, tag="t0")
                t1 = tp.tile([128, BL], F32, tag="t1")
                nc.vector.tensor_mul(t1[:, :], gates[1][:, :], c_prev[:, sl])
                nc.vector.tensor_mul(t0[:, :], gates[0][:, :], gates[2][:, :])
                nc.vector.tensor_add(c_new[:, sl], t0[:, :], t1[:, :])
                th = tp.tile([128, BL], F16, tag="th")
                nc.scalar.activation(th[:, :], c_new[:, sl], AF.Tanh)
                nc.vector.tensor_mul(h_new[:, sl], gates[3][:, :], th[:, :])
                nc.sync.dma_start(hs[t, hc], h_new[:, sl])

            # ---- xg phase: xg = W_ih^T x, streamed against the W_ih DMAs.
            # W_hh arrives during this phase and its tail overlaps step 0.
            for hc in range(NKH):
                mg = hc // 2
                mlo = 4 * (hc % 2)  # 0 or 4: this hc's cols in the wih tiles
                psx = [psump.tile([128, BL], F32, tag="ps", name="psx")
                       for _ in range(4)]
                for kc in range(NKX):
                    wt = wih_tiles[(mg, kc)]
                    for gi in range(4):
                        nc.tensor.matmul(
                            psx[gi][:, :],
                            wt[:, (mlo + gi) * 128:(mlo + gi + 1) * 128],
                            xt_sb[:, kc * BL:(kc + 1) * BL],
                            start=(kc == 0), stop=(kc == NKX - 1))
                # xg to SBUF (f16) on the DVE: the ACT engine must enter
                # step 0 without a copy backlog or its drains lag the PE
                for gi in range(4):
                    m = 4 * hc + gi
                    nc.vector.tensor_copy(xg_sb[:, m * BL:(m + 1) * BL],
                                          psx[gi][:, :])

            # ---- steps 0..T-1 ----
            for t in range(T):
                h_new = hp.tile([128, NKH * BL], F16, tag="h")
                c_new = cp.tile([128, NKH * BL], F32, tag="c")
                for hc in range(NKH):
                    ident_xg = t == T - 1 and hc == NKH - 1
                    ps = rec_matmuls(hc, h_prev, ident_xg)
                    drain_hc(t, hc, ps, h_new, c_new, ident_xg)
                h_prev, c_prev = h_new, c_new

    nc.compile()
    return nc


def timeline_ns():
    from concourse.timeline_sim import TimelineSim
    nc = _get_nc()
    ts = TimelineSim(nc)
    ts.simulate()
    return ts.time


def _get_nc():
    global _cached_nc
    if _cached_nc is None:
        _cached_nc = _build()
    return _cached_nc


def _perm():
    """Gate-row permutation: new position m*128+rr (m = hc*4+gi) <- original
    gate row gi*1024 + hc*128 + rr."""
    gi, hc, rr = np.meshgrid(np.arange(4), np.arange(NKH), np.arange(128),
                             indexing="ij")
    p = np.empty(G4, np.int64)
    m = hc * 4 + gi
    p[(m * 128 + rr).ravel()] = (gi * 1024 + hc * 128 + rr).ravel()
    return p


def make_inputs(x, W_ih, W_hh, b_ih, b_hh):
    """Host-side packing shared by kernel() and the quick tester."""
    f16 = np.float16
    perm = _perm()
    # W_ih^T cols permuted -> [16 kc, 128, 4 mg, 1024] -> [4, 16, 128, 1024]
    wihP = np.ascontiguousarray(
        W_ih.T[:, perm].reshape(NKX, 128, NMG, 1024).transpose(2, 0, 1, 3)
    ).astype(f16)
    whhP = np.ascontiguousarray(
        W_hh.T[:, perm].reshape(NKH, 128, NMG, 1024).transpose(2, 1, 0, 3)
    ).astype(f16)
    biasP = np.ascontiguousarray(
        (b_ih + b_hh)[perm].reshape(NM, 128).T).astype(np.float32)
    in_maps = []
    for c in range(NCORES):
        xtP = np.ascontiguousarray(
            x[c * BL:(c + 1) * BL].T.reshape(NKX, 128, BL).transpose(1, 0, 2)
        ).astype(f16)
        in_maps.append({"wih": wihP, "whh": whhP, "xt": xtP, "bias": biasP})
    return in_maps


def unpack_out(hs_f16):
    """[T, 8, 128, BL] f16 -> [T, BL, H] f32 for one core."""
    return hs_f16.transpose(0, 3, 1, 2).reshape(T, BL, H).astype(np.float32)


def kernel(x, W_ih, W_hh, b_ih, b_hh):
    global LAST_EXEC_NS, LAST_RESULTS
    nc = _get_nc()
    x = np.asarray(x, np.float32)
    in_maps = make_inputs(x, np.asarray(W_ih, np.float32),
                          np.asarray(W_hh, np.float32),
                          np.asarray(b_ih, np.float32),
                          np.asarray(b_hh, np.float32))
    trace = os.environ.get("LSTM_TRACE") == "1"
    res = run_bass_kernel_spmd(
        nc, in_maps, core_ids=list(range(NCORES)), trace=trace
    )
    LAST_EXEC_NS = res.exec_time_ns
    LAST_RESULTS = res

    out = np.empty((T, B, H), np.float32)
    for c in range(NCORES):
        out[:, c * BL:(c + 1) * BL, :] = unpack_out(res.results[c]["hs"])
    return out
